# revision 32
# baseline (speedup 1.0000x reference)
import sys
sys.path.insert(0, '/opt/trn_rl_repo')
"""Deformable-attention Bass kernel (one batch image per core).

Pipelined rewrite: single q GEMM (natural layout) + tensor-engine
transposes to build the bf16 gather map, and a fused attention+output
phase that software-pipelines sim -> exp -> AV -> transpose -> final
projection per 1024-column block so the PE array never idles.
"""
import numpy as np
import concourse.bass as bass
import concourse.tile as tile
from concourse import bacc, mybir

F32 = mybir.dt.float32
F32R = mybir.dt.float32r
BF16 = mybir.dt.bfloat16
I32 = mybir.dt.int32
AF = mybir.ActivationFunctionType
OP = mybir.AluOpType

DIM = 512; INNER = 512; H = W = 64; S = H * W
G = 8; D = 64; HEADS = 8; GH = GW = 16; J = GH * GW
SCALE = D ** -0.5
C15 = 64.0 / 15.0
MROWS = S + 2  # per-group map rows incl front/back guard


def host_constants():
    j_of = (np.arange(2)[None, :, None] * 128 + np.arange(128)[:, None, None])
    j_of = np.broadcast_to(j_of, (128, 2, 8)).reshape(128, 16)  # [p, m=t*8+g]
    meshA = (j_of // GW) * C15 - 0.5
    meshB = (j_of % GW) * C15 - 0.5
    return meshA.astype(np.float32), meshB.astype(np.float32)


def prep_weights(w_q, w_off1, b_off1, w_off2, w_kv, w_out, b_out):
    w_q = np.asarray(w_q, np.float32); w_kv = np.asarray(w_kv, np.float32)
    w_out = np.asarray(w_out, np.float32)
    W2 = np.zeros((2, INNER, G), np.float32)
    for g in range(G):
        for k in range(2):
            W2[k, g * D:(g + 1) * D, g] = np.asarray(w_off2, np.float32)[k]
    return {
        "w_qT": np.ascontiguousarray(w_q.T),
        "w_kT": np.ascontiguousarray(w_kv[:INNER].T),
        "w_vT": np.ascontiguousarray(w_kv[INNER:].T),
        "w_oT": np.ascontiguousarray(w_out.T),
        "w1v": np.ascontiguousarray(np.tile(np.asarray(w_off1, np.float32), G))[:, None],
        "b1v": np.ascontiguousarray(np.tile(np.asarray(b_off1, np.float32), G))[:, None],
        "W2x": np.ascontiguousarray(W2[0]),
        "W2y": np.ascontiguousarray(W2[1]),
        "b_out": np.asarray(b_out, np.float32)[:, None],
    }


def build(stage=5):
    nc = bacc.Bacc("TRN2", target_bir_lowering=False)
    x_in = nc.dram_tensor("x", [DIM, S], F32, kind="ExternalInput")
    w_qT = nc.dram_tensor("w_qT", [DIM, INNER], F32, kind="ExternalInput")
    w_kT = nc.dram_tensor("w_kT", [INNER, INNER], F32, kind="ExternalInput")
    w_vT = nc.dram_tensor("w_vT", [INNER, INNER], F32, kind="ExternalInput")
    w_oT = nc.dram_tensor("w_oT", [INNER, DIM], F32, kind="ExternalInput")
    w1v = nc.dram_tensor("w1v", [INNER, 1], F32, kind="ExternalInput")
    b1v = nc.dram_tensor("b1v", [INNER, 1], F32, kind="ExternalInput")
    W2x = nc.dram_tensor("W2x", [INNER, G], F32, kind="ExternalInput")
    W2y = nc.dram_tensor("W2y", [INNER, G], F32, kind="ExternalInput")
    b_out = nc.dram_tensor("b_out", [DIM, 1], F32, kind="ExternalInput")
    y_out = nc.dram_tensor("y", [DIM, S], F32, kind="ExternalOutput")

    meshA_np, meshB_np = host_constants()
    meshA_d = nc.inline_tensor(meshA_np, "meshA")
    meshB_d = nc.inline_tensor(meshB_np, "meshB")

    with tile.TileContext(nc) as tc:
        # ---------------- persistent pool ----------------
        P0 = tc.alloc_tile_pool(name="P0", bufs=1)
        ident = P0.tile([128, 128], F32)
        from concourse.masks import make_identity
        make_identity(nc, ident[:])
        ident_bf = P0.tile([128, 128], BF16)
        nc.vector.tensor_copy(ident_bf[:], ident[:])
        meshA = P0.tile([128, 16], F32); meshB = P0.tile([128, 16], F32)
        nc.sync.dma_start(meshA[:], meshA_d.ap())
        nc.sync.dma_start(meshB[:], meshB_d.ap())
        w1_sb = P0.tile([128, 4], F32); b1_sb = P0.tile([128, 4], F32)
        nc.sync.dma_start(w1_sb[:], w1v.ap().rearrange("(c p) one -> p (c one)", p=128))
        nc.sync.dma_start(b1_sb[:], b1v.ap().rearrange("(c p) one -> p (c one)", p=128))
        W2x_sb = P0.tile([128, 4, G], F32); W2y_sb = P0.tile([128, 4, G], F32)
        nc.sync.dma_start(W2x_sb[:], W2x.ap().rearrange("(c p) g -> p c g", p=128))
        nc.sync.dma_start(W2y_sb[:], W2y.ap().rearrange("(c p) g -> p c g", p=128))
        bout_sb = P0.tile([128, 4], F32)
        nc.sync.dma_start(bout_sb[:], b_out.ap().rearrange("(c p) one -> p (c one)", p=128))
        IDX = P0.tile([128, 32], I32)
        Wb = P0.tile([128, 64], F32)
        kvf = P0.tile([128, 4, J], BF16)
        k_sb = P0.tile([128, 4, J], BF16)
        vT_sb = P0.tile([128, 2, 8 * 65], BF16)
        wo_sb = P0.tile([128, 4, DIM], BF16)
        for c in range(4):
            nc.gpsimd.dma_start(wo_sb[:, c, :], w_oT.ap()[c * 128:(c + 1) * 128, :])
        wkv_pool = tc.alloc_tile_pool(name="wkvp", bufs=1)
        wk_sb = wkv_pool.tile([128, 4, INNER], BF16)
        wv_sb = wkv_pool.tile([128, 4, INNER], BF16)
        for c in range(4):
            nc.gpsimd.dma_start(wk_sb[:, c, :], w_kT.ap()[c * 128:(c + 1) * 128, :])
            nc.gpsimd.dma_start(wv_sb[:, c, :], w_vT.ap()[c * 128:(c + 1) * 128, :])

        q_pool = tc.alloc_tile_pool(name="qp", bufs=1)
        q_sb = q_pool.tile([128, 4, S], BF16)

        # DRAM scratch: per-group transposed-q map (bf16) for the gathers
        drp = tc.alloc_tile_pool(name="dr", bufs=1, space="DRAM")
        qt_map = drp.tile([G * MROWS, D], F32)
        zt = P0.tile([G, 2, D], F32)
        nc.vector.memset(zt[:], 0.0)
        guard_dst = bass.AP(tensor=qt_map[:].tensor, offset=qt_map[:].offset,
                            ap=[[MROWS * D, G], [(MROWS - 1) * D, 2], [1, D]])
        nc.sync.dma_start(guard_dst, zt[:])

        # ---------------- phase A: x load + natural-q GEMM + qT ----------------
        wq_pool = tc.alloc_tile_pool(name="wqp", bufs=1)
        wq_sb = wq_pool.tile([128, 4, INNER], F32R)
        for c in range(4):
            nc.gpsimd.dma_start(wq_sb[:, c, :], w_qT.ap()[c * 128:(c + 1) * 128, :])
        x_pool = tc.alloc_tile_pool(name="xp", bufs=1)
        x_sb = x_pool.tile([128, 4, S], F32R)
        for q4 in range(4):
            for c in range(4):
                nc.gpsimd.dma_start(
                    x_sb[:, c, q4 * 1024:(q4 + 1) * 1024],
                    x_in.ap()[c * 128:(c + 1) * 128, q4 * 1024:(q4 + 1) * 1024])

        # Dependency absorption: a matmul's LDWEIGHTS slot allows only ONE
        # sync wait, so a matmul whose operands arrive via two different DMA
        # queues fails walrus codegen. Funnel every weight-DMA dependency
        # through tiny vector copies into a bf16 staging tile, then order the
        # tensor queue after all of them with a single warm matmul.
        warm_pool = tc.alloc_tile_pool(name="warm", bufs=1, space="PSUM")
        warm_t = warm_pool.tile([32, 32], F32)
        stg = P0.tile([128, 32], BF16)
        _wi = [0]

        def vdep(ap2):
            if ap2.dtype == F32R:
                ap2 = ap2.bitcast(F32)
            nc.vector.tensor_copy(stg[:, _wi[0]:_wi[0] + 1], ap2)
            _wi[0] += 1

        for c in range(4):
            vdep(wq_sb[:, c, 0:1])
            vdep(wk_sb[:, c, 0:1])
            vdep(wv_sb[:, c, 0:1])
            vdep(wo_sb[:, c, 0:1])
        vdep(ident[:, 0:1])
        vdep(ident_bf[:, 0:1])
        nc.tensor.matmul(warm_t[0:_wi[0], 0:_wi[0]], stg[:, 0:_wi[0]],
                         stg[:, 0:_wi[0]], start=True, stop=True)

        psA = tc.alloc_tile_pool(name="psA", bufs=3, space="PSUM")
        psQT = tc.alloc_tile_pool(name="psQT", bufs=2, space="PSUM")
        qts_pool = tc.alloc_tile_pool(name="qts", bufs=3)
        for sb in range(8):
            s0 = sb * 512
            for oc in range(4):
                pq = psA.tile([128, 512], F32, tag="pq")
                for c in range(4):
                    nc.tensor.matmul(pq[:], wq_sb[:, c, oc * 128:(oc + 1) * 128],
                                     x_sb[:, c, s0:s0 + 512],
                                     start=(c == 0), stop=(c == 3))
                nc.vector.tensor_copy(q_sb[:, oc, s0:s0 + 512], pq[:])
            # transpose this 512-col block into qt_map rows (4 chunks of 128 s)
            for ch in range(4):
                cs = s0 + ch * 128
                ptq = psQT.tile([128, 512], BF16, tag="ptq")
                for oc in range(4):
                    nc.tensor.transpose(ptq[:, oc * 128:(oc + 1) * 128],
                                        q_sb[:, oc, cs:cs + 128], ident_bf[:])
                qts = qts_pool.tile([128, 8, 64], F32, tag="qts")
                nc.vector.tensor_copy(qts[:], ptq[:].rearrange("p (g d) -> p g d", g=G))
                for g in range(G):
                    r0 = g * MROWS + 1 + cs
                    nc.sync.dma_start(qt_map[r0:r0 + 128, :], qts[:, g, :])

        # ---------------- offsets ----------------
        offp = tc.alloc_tile_pool(name="offp", bufs=1)
        psOff = tc.alloc_tile_pool(name="psOff", bufs=1, space="PSUM")
        t_sb = offp.tile([128, 4, J], F32)
        for ic in range(4):
            pqd = psOff.tile([128, J], F32, tag="pqd")
            for c in range(4):
                base = x_sb[:, c, :]
                rhs = bass.AP(tensor=base.tensor, offset=base.offset,
                              ap=[list(base.ap[0]), [256, 16], [4, 16]])
                nc.tensor.matmul(pqd[:], wq_sb[:, c, ic * 128:(ic + 1) * 128], rhs,
                                 start=(c == 0), stop=(c == 3))
            nc.scalar.activation(t_sb[:, ic, :], pqd[:], AF.Gelu,
                                 bias=b1_sb[:, ic:ic + 1], scale=w1_sb[:, ic:ic + 1])
        offx = offp.tile([128, 16], F32); offy = offp.tile([128, 16], F32)
        for jt in range(2):
            pxy = psOff.tile([128, 2, G], F32, tag="pxy")
            px = pxy[:, 0, :]; py = pxy[:, 1, :]
            for c in range(4):
                nc.tensor.matmul(px, t_sb[:, c, jt * 128:(jt + 1) * 128],
                                 W2x_sb[:, c, :], start=(c == 0), stop=(c == 3))
            for c in range(4):
                nc.tensor.matmul(py, t_sb[:, c, jt * 128:(jt + 1) * 128],
                                 W2y_sb[:, c, :], start=(c == 0), stop=(c == 3))
            nc.scalar.activation(offx[:, jt * 8:(jt + 1) * 8], px, AF.Tanh)
            nc.scalar.activation(offy[:, jt * 8:(jt + 1) * 8], py, AF.Tanh)

        _fc = [0]
        def f16():
            _fc[0] += 1
            return offp.tile([128, 16], F32, name=f"f16_{_fc[0]}", tag=f"f16_{_fc[0]}")

        xs = f16(); ys = f16()
        nc.vector.scalar_tensor_tensor(out=xs[:], in0=offx[:], scalar=4.0 * C15,
                                       in1=meshA[:], op0=OP.mult, op1=OP.add)
        nc.vector.scalar_tensor_tensor(out=ys[:], in0=offy[:], scalar=4.0 * C15,
                                       in1=meshB[:], op0=OP.mult, op1=OP.add)

        def floor_of(src):
            _fc[0] += 1
            ti = offp.tile([128, 16], I32, name=f"i16_{_fc[0]}", tag=f"i16_{_fc[0]}")
            nc.vector.tensor_copy(ti[:], src)
            tf = f16()
            nc.vector.tensor_copy(tf[:], ti[:])
            gt = f16()
            nc.vector.tensor_tensor(out=gt[:], in0=tf[:], in1=src, op=OP.is_gt)
            fl = f16()
            nc.vector.tensor_tensor(out=fl[:], in0=tf[:], in1=gt[:], op=OP.subtract)
            return fl

        x0f = floor_of(xs[:]); y0f = floor_of(ys[:])

        def in_range(v, lo, hi):
            a = f16(); b2 = f16(); r = f16()
            nc.vector.tensor_scalar(out=a[:], in0=v, scalar1=float(lo), scalar2=None,
                                    op0=OP.is_ge)
            nc.vector.tensor_scalar(out=b2[:], in0=v, scalar1=float(hi), scalar2=None,
                                    op0=OP.is_le)
            nc.vector.tensor_tensor(out=r[:], in0=a[:], in1=b2[:], op=OP.mult)
            return r

        vx0 = in_range(x0f[:], 0, 63); vx1 = in_range(x0f[:], -1, 62)
        vy0 = in_range(y0f[:], 0, 63); vy1 = in_range(y0f[:], -1, 62)
        wx1 = f16(); wy1 = f16()
        nc.vector.tensor_tensor(out=wx1[:], in0=xs[:], in1=x0f[:], op=OP.subtract)
        nc.vector.tensor_tensor(out=wy1[:], in0=ys[:], in1=y0f[:], op=OP.subtract)
        wx0m = f16(); wx1m = f16(); wy0m = f16(); wy1m = f16()
        nc.vector.scalar_tensor_tensor(out=wx0m[:], in0=wx1[:], scalar=1.0,
                                       in1=vx0[:], op0=OP.subtract, op1=OP.mult)
        nc.vector.tensor_scalar_mul(wx0m[:], wx0m[:], -1.0)
        nc.vector.tensor_tensor(out=wx1m[:], in0=wx1[:], in1=vx1[:], op=OP.mult)
        nc.vector.scalar_tensor_tensor(out=wy0m[:], in0=wy1[:], scalar=1.0,
                                       in1=vy0[:], op0=OP.subtract, op1=OP.mult)
        nc.vector.tensor_scalar_mul(wy0m[:], wy0m[:], -1.0)
        nc.vector.tensor_tensor(out=wy1m[:], in0=wy1[:], in1=vy1[:], op=OP.mult)
        nc.vector.tensor_tensor(out=Wb[:, 0:16], in0=wy0m[:], in1=wx0m[:], op=OP.mult)
        nc.vector.tensor_tensor(out=Wb[:, 16:32], in0=wy0m[:], in1=wx1m[:], op=OP.mult)
        nc.vector.tensor_tensor(out=Wb[:, 32:48], in0=wy1m[:], in1=wx0m[:], op=OP.mult)
        nc.vector.tensor_tensor(out=Wb[:, 48:64], in0=wy1m[:], in1=wx1m[:], op=OP.mult)
        xm = f16(); ym0 = f16(); ym1 = f16()
        nc.vector.tensor_scalar(out=xm[:], in0=x0f[:], scalar1=-1.0, scalar2=63.0,
                                op0=OP.max, op1=OP.min)
        nc.vector.tensor_scalar(out=ym0[:], in0=y0f[:], scalar1=0.0, scalar2=63.0,
                                op0=OP.max, op1=OP.min)
        nc.vector.tensor_scalar(out=ym1[:], in0=y0f[:], scalar1=1.0, scalar2=0.0,
                                op0=OP.add, op1=OP.max)
        nc.vector.tensor_scalar_min(ym1[:], ym1[:], 63.0)
        IDXf = offp.tile([128, 32], F32)
        nc.vector.scalar_tensor_tensor(out=IDXf[:, 0:16], in0=ym0[:], scalar=64.0,
                                       in1=xm[:], op0=OP.mult, op1=OP.add)
        nc.vector.scalar_tensor_tensor(out=IDXf[:, 16:32], in0=ym1[:], scalar=64.0,
                                       in1=xm[:], op0=OP.mult, op1=OP.add)
        nc.vector.tensor_copy(IDX[:], IDXf[:])

        psOff.release(); offp.release()
        qts_pool.release(); psQT.release(); psA.release()
        if stage < 2:
            warm_pool.release(); x_pool.release(); wq_pool.release()
            drp.release(); q_pool.release(); wkv_pool.release(); P0.release()
            nc.compile(); return nc

        # ---------------- gathers + bilinear + kvf ----------------
        gpool = tc.alloc_tile_pool(name="gp", bufs=3)
        psT = tc.alloc_tile_pool(name="psT", bufs=2, space="PSUM")
        qt_flat = qt_map[:]
        for g in range(G):
            Gt = gpool.tile([128, 512], F32, tag="G")
            for yy in range(2):
                for t in range(2):
                    col = yy * 16 + t * 8 + g
                    nc.gpsimd.indirect_dma_start(
                        out=Gt[:, (yy * 2 + t) * 128:(yy * 2 + t + 1) * 128],
                        out_offset=None, in_=qt_flat,
                        in_offset=bass.IndirectOffsetOnAxis(
                            ap=IDX[:, col:col + 1], axis=0),
                        element_offset=(g * MROWS + 1) * D)
            for t in range(2):
                acc = gpool.tile([128, D], F32, tag="acc")
                m = t * 8 + g
                nc.vector.tensor_scalar(out=acc[:], in0=Gt[:, t * 128:t * 128 + 64],
                                        scalar1=Wb[:, m:m + 1], scalar2=None,
                                        op0=OP.mult)
                for yy, xx in ((0, 1), (1, 0), (1, 1)):
                    blk = (yy * 2 + t) * 128 + xx * 64
                    wcol = (2 * yy + xx) * 16 + m
                    nc.vector.scalar_tensor_tensor(
                        out=acc[:], in0=Gt[:, blk:blk + 64],
                        scalar=Wb[:, wcol:wcol + 1], in1=acc[:],
                        op0=OP.mult, op1=OP.add)
                accb = gpool.tile([128, D], BF16, tag="accb")
                nc.vector.tensor_copy(accb[:], acc[:])
                pt = psT.tile([64, 128], BF16, tag="pt")
                nc.tensor.transpose(pt[:], accb[:], ident_bf[:])
                nc.vector.tensor_copy(
                    kvf[(g % 2) * 64:(g % 2) * 64 + 64, g // 2, t * 128:(t + 1) * 128],
                    pt[:])

        if stage < 3:
            psT.release(); gpool.release()
            warm_pool.release(); x_pool.release(); wq_pool.release()
            drp.release(); q_pool.release(); wkv_pool.release(); P0.release()
            nc.compile(); return nc
        # ---------------- k and vT ----------------
        psKV = tc.alloc_tile_pool(name="psKV", bufs=2, space="PSUM")
        for oc in range(4):
            pk = psKV.tile([128, J], F32, tag="pk")
            for c in range(4):
                nc.tensor.matmul(pk[:], wk_sb[:, c, oc * 128:(oc + 1) * 128],
                                 kvf[:, c, :], start=(c == 0), stop=(c == 3))
            nc.vector.tensor_copy(k_sb[:, oc, :], pk[:])
        nc.vector.memset(vT_sb[:], 1.0)
        for jt in range(2):
            pv = psKV.tile([128, INNER], F32, tag="pv")
            for c in range(4):
                nc.tensor.matmul(pv[:], kvf[:, c, jt * 128:(jt + 1) * 128],
                                 wv_sb[:, c, :], start=(c == 0), stop=(c == 3))
            for h in range(8):
                nc.vector.tensor_copy(vT_sb[:, jt, h * 65:h * 65 + 64],
                                      pv[:, h * 64:(h + 1) * 64])
        i0 = _wi[0]
        vdep(k_sb[:, 0, 0:1])
        vdep(vT_sb[:, 0, 0:1])
        nc.tensor.matmul(warm_t[0:2, 24:26], stg[:, i0:i0 + 2],
                         stg[:, i0:i0 + 2], start=True, stop=True)
        psKV.release(); psT.release(); gpool.release()
        warm_pool.release()
        x_pool.release(); wq_pool.release()
        if stage < 4:
            drp.release(); q_pool.release(); wkv_pool.release(); P0.release()
            nc.compile(); return nc

        # ---------------- fused attention + output projection ----------------
        # PSUM budget (8 banks): psS 2 bufs x [128,2,512]f32 (2 banks each) = 4,
        # psAV 2 x [128,4,65]f32 = 2, psTB 1 x [128,512]bf16 = 1,
        # psF 1 x [128,512]f32 = 1.
        outT_pool = tc.alloc_tile_pool(name="otp", bufs=1)
        outT_sb = outT_pool.tile([128, 4, S], BF16)
        ep = tc.alloc_tile_pool(name="ep", bufs=3)
        psS = tc.alloc_tile_pool(name="psS", bufs=2, space="PSUM")
        psAV = tc.alloc_tile_pool(name="psAV", bufs=2, space="PSUM")
        psTB = tc.alloc_tile_pool(name="psTB", bufs=1, space="PSUM")
        psF = tc.alloc_tile_pool(name="psF", bufs=1, space="PSUM")
        nrm = tc.alloc_tile_pool(name="nrm", bufs=3)
        recp = tc.alloc_tile_pool(name="recp", bufs=4)
        yev = tc.alloc_tile_pool(name="yev", bufs=2)

        E_tiles = {}
        outF_tiles = {}

        def emit_sim(u):
            """sim + exp for unit u = (blk, hp): 1024 cols of E for 2 heads."""
            blk, hp = u // 4, u % 4
            bs = blk * 1024
            E = ep.tile([128, 2, 2, 1024], BF16, tag="E")
            E_tiles[u] = E
            for jt in range(2):
                for half in range(2):
                    cs = bs + half * 512
                    ps = psS.tile([128, 2, 512], F32, tag="ps")
                    for hh in range(2):
                        nc.tensor.matmul(
                            ps[:, hh, :],
                            k_sb[hh * 64:(hh + 1) * 64, hp, jt * 128:(jt + 1) * 128],
                            q_sb[hh * 64:(hh + 1) * 64, hp, cs:cs + 512],
                            start=True, stop=True)
                    nc.scalar.activation(
                        E[:, :, jt, half * 512:(half + 1) * 512], ps[:],
                        AF.Exp, scale=SCALE)

        def emit_av_mm(u):
            """AV matmuls + normalize into outF tiles for unit u."""
            blk, hp = u // 4, u % 4
            E = E_tiles.pop(u)
            for sl in range(2):  # 512-col halves of the block
                outF = nrm.tile([128, 4, 128], BF16, tag="outF")
                outF_tiles[(u, sl)] = outF
                for hh in range(2):
                    h = 2 * hp + hh
                    pav = psAV.tile([128, 4, 65], F32, tag="pav")
                    for ck in range(4):
                        cl = sl * 512 + ck * 128
                        for jt in range(2):
                            nc.tensor.matmul(
                                pav[:, ck, :], E[:, hh, jt, cl:cl + 128],
                                vT_sb[:, jt, h * 65:(h + 1) * 65],
                                start=(jt == 0), stop=(jt == 1))
                    rec = recp.tile([128, 4], F32, tag="rec")
                    nc.vector.reciprocal(rec[:], pav[:, :, 64])
                    nc.vector.tensor_tensor(
                        out=outF[:, :, hh * 64:(hh + 1) * 64],
                        in0=pav[:, :, 0:64],
                        in1=rec[:].unsqueeze(-1).to_broadcast((128, 4, 64)),
                        op=OP.mult)

        def emit_ptb(u):
            """transpose outF into outT columns for unit u."""
            blk, hp = u // 4, u % 4
            for sl in range(2):
                sb2 = blk * 2 + sl
                outF = outF_tiles.pop((u, sl))
                ptb = psTB.tile([128, 512], BF16, tag="ptb")
                for ck in range(4):
                    nc.tensor.transpose(ptb[:, ck * 128:(ck + 1) * 128],
                                        outF[:, ck, :], ident_bf[:])
                nc.vector.tensor_copy(
                    outT_sb[:, hp, sb2 * 512:(sb2 + 1) * 512], ptb[:])

        def emit_final_chunk(sb2, oc):
            pf = psF.tile([128, 512], F32, tag="pf")
            for ic in range(4):
                nc.tensor.matmul(pf[:], wo_sb[:, ic, oc * 128:(oc + 1) * 128],
                                 outT_sb[:, ic, sb2 * 512:(sb2 + 1) * 512],
                                 start=(ic == 0), stop=(ic == 3))
            ye = yev.tile([128, 512], F32, tag="ye")
            nc.vector.tensor_scalar(out=ye[:], in0=pf[:],
                                    scalar1=bout_sb[:, oc:oc + 1], scalar2=None,
                                    op0=OP.add)
            nc.sync.dma_start(
                y_out.ap()[oc * 128:(oc + 1) * 128,
                           sb2 * 512:(sb2 + 1) * 512], ye[:])

        # software pipeline over 16 (blk, hp) units; final-projection chunks
        # of a completed block are drip-fed between later units so the
        # single-buffered psF never stalls the tensor queue.
        pending_final = []
        emit_sim(0)
        emit_sim(1)
        for u in range(16):
            emit_av_mm(u)
            if u + 2 < 16:
                emit_sim(u + 2)
            emit_ptb(u)
            if u % 4 == 3:
                blk = u // 4
                pending_final.extend(
                    (blk * 2 + sl, oc) for sl in range(2) for oc in range(4))
            for _ in range(2):
                if pending_final and u < 15:
                    emit_final_chunk(*pending_final.pop(0))
        while pending_final:
            emit_final_chunk(*pending_final.pop(0))

        yev.release(); recp.release(); nrm.release()
        psF.release(); psTB.release(); psAV.release(); psS.release(); ep.release()
        outT_pool.release()
        drp.release(); q_pool.release(); wkv_pool.release(); P0.release()
    nc.compile()
    return nc


# ---------------------------------------------------------------------------
# Public entry point: full (unsharded) inputs -> full output.
# Data-parallel over batch: image i runs on NeuronCore i (8 cores).
# ---------------------------------------------------------------------------
_NC_CACHE = {}


def _get_nc():
    if "nc" not in _NC_CACHE:
        _NC_CACHE["nc"] = build()
    return _NC_CACHE["nc"]


def kernel(x, w_q, w_off1, b_off1, w_off2, w_kv, w_out, b_out):
    from concourse.bass_utils import run_bass_kernel_spmd
    x = np.asarray(x, np.float32)
    b = x.shape[0]
    assert x.shape == (8, DIM, H, W), f"unexpected x shape {x.shape}"
    wd = prep_weights(w_q, w_off1, b_off1, w_off2, w_kv, w_out, b_out)
    in_maps = [{"x": np.ascontiguousarray(x[i].reshape(DIM, S)), **wd}
               for i in range(b)]
    nc = _get_nc()
    res = run_bass_kernel_spmd(nc, in_maps, core_ids=list(range(b)))
    out = np.stack([res.results[i]["y"].reshape(DIM, H, W) for i in range(b)])
    return out.astype(np.float32)


# revision 33
# speedup vs baseline: 1.2361x; 1.2361x over previous
import sys
sys.path.insert(0, '/opt/trn_rl_repo')
"""Deformable-attention Bass kernel (one batch image per core).

Pipelined rewrite: single q GEMM (natural layout) + tensor-engine
transposes to build the bf16 gather map, and a fused attention+output
phase that software-pipelines sim -> exp -> AV -> transpose -> final
projection per 1024-column block so the PE array never idles.
"""
import numpy as np
import concourse.bass as bass
import concourse.tile as tile
from concourse import bacc, mybir

F32 = mybir.dt.float32
F32R = mybir.dt.float32r
BF16 = mybir.dt.bfloat16
I32 = mybir.dt.int32
AF = mybir.ActivationFunctionType
OP = mybir.AluOpType

DIM = 512; INNER = 512; H = W = 64; S = H * W
G = 8; D = 64; HEADS = 8; GH = GW = 16; J = GH * GW
SCALE = D ** -0.5
C15 = 64.0 / 15.0
MROWS = S + 2  # per-group map rows incl front/back guard


def host_constants():
    j_of = (np.arange(2)[None, :, None] * 128 + np.arange(128)[:, None, None])
    j_of = np.broadcast_to(j_of, (128, 2, 8)).reshape(128, 16)  # [p, m=t*8+g]
    meshA = (j_of // GW) * C15 - 0.5
    meshB = (j_of % GW) * C15 - 0.5
    return meshA.astype(np.float32), meshB.astype(np.float32)


def prep_weights(w_q, w_off1, b_off1, w_off2, w_kv, w_out, b_out):
    w_q = np.asarray(w_q, np.float32); w_kv = np.asarray(w_kv, np.float32)
    w_out = np.asarray(w_out, np.float32)
    W2 = np.zeros((2, INNER, G), np.float32)
    for g in range(G):
        for k in range(2):
            W2[k, g * D:(g + 1) * D, g] = np.asarray(w_off2, np.float32)[k]
    return {
        "w_qT": np.ascontiguousarray(w_q.T),
        "w_kT": np.ascontiguousarray(w_kv[:INNER].T),
        "w_vT": np.ascontiguousarray(w_kv[INNER:].T),
        "w_oT": np.ascontiguousarray(w_out.T),
        "w1v": np.ascontiguousarray(np.tile(np.asarray(w_off1, np.float32), G))[:, None],
        "b1v": np.ascontiguousarray(np.tile(np.asarray(b_off1, np.float32), G))[:, None],
        "W2x": np.ascontiguousarray(W2[0]),
        "W2y": np.ascontiguousarray(W2[1]),
        "b_out": np.asarray(b_out, np.float32)[:, None],
    }


def build(stage=5):
    nc = bacc.Bacc("TRN2", target_bir_lowering=False)
    x_in = nc.dram_tensor("x", [DIM, S], F32, kind="ExternalInput")
    w_qT = nc.dram_tensor("w_qT", [DIM, INNER], F32, kind="ExternalInput")
    w_kT = nc.dram_tensor("w_kT", [INNER, INNER], F32, kind="ExternalInput")
    w_vT = nc.dram_tensor("w_vT", [INNER, INNER], F32, kind="ExternalInput")
    w_oT = nc.dram_tensor("w_oT", [INNER, DIM], F32, kind="ExternalInput")
    w1v = nc.dram_tensor("w1v", [INNER, 1], F32, kind="ExternalInput")
    b1v = nc.dram_tensor("b1v", [INNER, 1], F32, kind="ExternalInput")
    W2x = nc.dram_tensor("W2x", [INNER, G], F32, kind="ExternalInput")
    W2y = nc.dram_tensor("W2y", [INNER, G], F32, kind="ExternalInput")
    b_out = nc.dram_tensor("b_out", [DIM, 1], F32, kind="ExternalInput")
    y_out = nc.dram_tensor("y", [DIM, S], F32, kind="ExternalOutput")

    meshA_np, meshB_np = host_constants()
    meshA_d = nc.inline_tensor(meshA_np, "meshA")
    meshB_d = nc.inline_tensor(meshB_np, "meshB")

    with tile.TileContext(nc) as tc:
        # ---------------- persistent pool ----------------
        P0 = tc.alloc_tile_pool(name="P0", bufs=1)
        ident = P0.tile([128, 128], F32)
        from concourse.masks import make_identity
        make_identity(nc, ident[:])
        ident_bf = P0.tile([128, 128], BF16)
        nc.vector.tensor_copy(ident_bf[:], ident[:])
        meshA = P0.tile([128, 16], F32); meshB = P0.tile([128, 16], F32)
        nc.sync.dma_start(meshA[:], meshA_d.ap())
        nc.sync.dma_start(meshB[:], meshB_d.ap())
        w1_sb = P0.tile([128, 4], F32); b1_sb = P0.tile([128, 4], F32)
        nc.sync.dma_start(w1_sb[:], w1v.ap().rearrange("(c p) one -> p (c one)", p=128))
        nc.sync.dma_start(b1_sb[:], b1v.ap().rearrange("(c p) one -> p (c one)", p=128))
        W2x_sb = P0.tile([128, 4, G], F32); W2y_sb = P0.tile([128, 4, G], F32)
        nc.sync.dma_start(W2x_sb[:], W2x.ap().rearrange("(c p) g -> p c g", p=128))
        nc.sync.dma_start(W2y_sb[:], W2y.ap().rearrange("(c p) g -> p c g", p=128))
        bout_sb = P0.tile([128, 4], F32)
        nc.sync.dma_start(bout_sb[:], b_out.ap().rearrange("(c p) one -> p (c one)", p=128))
        IDX = P0.tile([128, 32], I32)
        Wb = P0.tile([128, 64], F32)
        kvf = P0.tile([128, 4, J], BF16)
        k_sb = P0.tile([128, 4, J], BF16)
        vT_sb = P0.tile([128, 2, 8 * 65], BF16)
        wo_sb = P0.tile([128, 4, DIM], BF16)
        for c in range(4):
            nc.gpsimd.dma_start(wo_sb[:, c, :], w_oT.ap()[c * 128:(c + 1) * 128, :])
        wkv_pool = tc.alloc_tile_pool(name="wkvp", bufs=1)
        wk_sb = wkv_pool.tile([128, 4, INNER], BF16)
        wv_sb = wkv_pool.tile([128, 4, INNER], BF16)
        for c in range(4):
            nc.gpsimd.dma_start(wk_sb[:, c, :], w_kT.ap()[c * 128:(c + 1) * 128, :])
            nc.gpsimd.dma_start(wv_sb[:, c, :], w_vT.ap()[c * 128:(c + 1) * 128, :])

        q_pool = tc.alloc_tile_pool(name="qp", bufs=1)
        q_sb = q_pool.tile([128, 4, S], BF16)

        # DRAM scratch: per-group transposed-q map (bf16) for the gathers
        drp = tc.alloc_tile_pool(name="dr", bufs=1, space="DRAM")
        qt_map = drp.tile([G * MROWS, D], F32)
        zt = P0.tile([G, 2, D], F32)
        nc.vector.memset(zt[:], 0.0)
        guard_dst = bass.AP(tensor=qt_map[:].tensor, offset=qt_map[:].offset,
                            ap=[[MROWS * D, G], [(MROWS - 1) * D, 2], [1, D]])
        nc.sync.dma_start(guard_dst, zt[:])

        # ---------------- phase A: x load + natural-q GEMM + qT ----------------
        wq_pool = tc.alloc_tile_pool(name="wqp", bufs=1)
        wq_sb = wq_pool.tile([128, 4, INNER], F32R)
        for c in range(4):
            nc.gpsimd.dma_start(wq_sb[:, c, :], w_qT.ap()[c * 128:(c + 1) * 128, :])
        x_pool = tc.alloc_tile_pool(name="xp", bufs=1)
        x_sb = x_pool.tile([128, 4, S], F32R)
        for q4 in range(4):
            for c in range(4):
                nc.gpsimd.dma_start(
                    x_sb[:, c, q4 * 1024:(q4 + 1) * 1024],
                    x_in.ap()[c * 128:(c + 1) * 128, q4 * 1024:(q4 + 1) * 1024])

        # Dependency absorption: a matmul's LDWEIGHTS slot allows only ONE
        # sync wait, so a matmul whose operands arrive via two different DMA
        # queues fails walrus codegen. Funnel every weight-DMA dependency
        # through tiny vector copies into a bf16 staging tile, then order the
        # tensor queue after all of them with a single warm matmul.
        warm_pool = tc.alloc_tile_pool(name="warm", bufs=1, space="PSUM")
        warm_t = warm_pool.tile([32, 32], F32)
        stg = P0.tile([128, 32], BF16)
        _wi = [0]

        def vdep(ap2):
            if ap2.dtype == F32R:
                ap2 = ap2.bitcast(F32)
            nc.vector.tensor_copy(stg[:, _wi[0]:_wi[0] + 1], ap2)
            _wi[0] += 1

        for c in range(4):
            vdep(wq_sb[:, c, 0:1])
            vdep(wk_sb[:, c, 0:1])
            vdep(wv_sb[:, c, 0:1])
            vdep(wo_sb[:, c, 0:1])
        vdep(ident[:, 0:1])
        vdep(ident_bf[:, 0:1])
        nc.tensor.matmul(warm_t[0:_wi[0], 0:_wi[0]], stg[:, 0:_wi[0]],
                         stg[:, 0:_wi[0]], start=True, stop=True)

        psA = tc.alloc_tile_pool(name="psA", bufs=3, space="PSUM")
        psQT = tc.alloc_tile_pool(name="psQT", bufs=2, space="PSUM")
        qts_pool = tc.alloc_tile_pool(name="qts", bufs=3)
        for sb in range(8):
            s0 = sb * 512
            for oc in range(4):
                pq = psA.tile([128, 512], F32, tag="pq")
                for c in range(4):
                    nc.tensor.matmul(pq[:], wq_sb[:, c, oc * 128:(oc + 1) * 128],
                                     x_sb[:, c, s0:s0 + 512],
                                     start=(c == 0), stop=(c == 3))
                nc.vector.tensor_copy(q_sb[:, oc, s0:s0 + 512], pq[:])
            # transpose this 512-col block into qt_map rows (4 chunks of 128 s)
            for ch in range(4):
                cs = s0 + ch * 128
                ptq = psQT.tile([128, 512], BF16, tag="ptq")
                for oc in range(4):
                    nc.tensor.transpose(ptq[:, oc * 128:(oc + 1) * 128],
                                        q_sb[:, oc, cs:cs + 128], ident_bf[:])
                qts = qts_pool.tile([128, 512], F32, tag="qts")
                nc.vector.tensor_copy(qts[:], ptq[:])
                dst = bass.AP(tensor=qt_map[:].tensor,
                              offset=qt_map[:].offset + (1 + cs) * D,
                              ap=[[D, 128], [MROWS * D, G], [1, D]])
                nc.sync.dma_start(dst, qts[:].rearrange("p (g d) -> p g d", g=G))

        # ---------------- offsets ----------------
        offp = tc.alloc_tile_pool(name="offp", bufs=1)
        psOff = tc.alloc_tile_pool(name="psOff", bufs=1, space="PSUM")
        t_sb = offp.tile([128, 4, J], F32)
        for ic in range(4):
            pqd = psOff.tile([128, J], F32, tag="pqd")
            for c in range(4):
                base = x_sb[:, c, :]
                rhs = bass.AP(tensor=base.tensor, offset=base.offset,
                              ap=[list(base.ap[0]), [256, 16], [4, 16]])
                nc.tensor.matmul(pqd[:], wq_sb[:, c, ic * 128:(ic + 1) * 128], rhs,
                                 start=(c == 0), stop=(c == 3))
            nc.scalar.activation(t_sb[:, ic, :], pqd[:], AF.Gelu,
                                 bias=b1_sb[:, ic:ic + 1], scale=w1_sb[:, ic:ic + 1])
        offx = offp.tile([128, 16], F32); offy = offp.tile([128, 16], F32)
        for jt in range(2):
            pxy = psOff.tile([128, 2, G], F32, tag="pxy")
            px = pxy[:, 0, :]; py = pxy[:, 1, :]
            for c in range(4):
                nc.tensor.matmul(px, t_sb[:, c, jt * 128:(jt + 1) * 128],
                                 W2x_sb[:, c, :], start=(c == 0), stop=(c == 3))
            for c in range(4):
                nc.tensor.matmul(py, t_sb[:, c, jt * 128:(jt + 1) * 128],
                                 W2y_sb[:, c, :], start=(c == 0), stop=(c == 3))
            nc.scalar.activation(offx[:, jt * 8:(jt + 1) * 8], px, AF.Tanh)
            nc.scalar.activation(offy[:, jt * 8:(jt + 1) * 8], py, AF.Tanh)

        _fc = [0]
        def f16():
            _fc[0] += 1
            return offp.tile([128, 16], F32, name=f"f16_{_fc[0]}", tag=f"f16_{_fc[0]}")

        xs = f16(); ys = f16()
        nc.vector.scalar_tensor_tensor(out=xs[:], in0=offx[:], scalar=4.0 * C15,
                                       in1=meshA[:], op0=OP.mult, op1=OP.add)
        nc.vector.scalar_tensor_tensor(out=ys[:], in0=offy[:], scalar=4.0 * C15,
                                       in1=meshB[:], op0=OP.mult, op1=OP.add)

        def floor_of(src):
            _fc[0] += 1
            ti = offp.tile([128, 16], I32, name=f"i16_{_fc[0]}", tag=f"i16_{_fc[0]}")
            nc.vector.tensor_copy(ti[:], src)
            tf = f16()
            nc.vector.tensor_copy(tf[:], ti[:])
            gt = f16()
            nc.vector.tensor_tensor(out=gt[:], in0=tf[:], in1=src, op=OP.is_gt)
            fl = f16()
            nc.vector.tensor_tensor(out=fl[:], in0=tf[:], in1=gt[:], op=OP.subtract)
            return fl

        x0f = floor_of(xs[:]); y0f = floor_of(ys[:])

        def in_range(v, lo, hi):
            a = f16(); b2 = f16(); r = f16()
            nc.vector.tensor_scalar(out=a[:], in0=v, scalar1=float(lo), scalar2=None,
                                    op0=OP.is_ge)
            nc.vector.tensor_scalar(out=b2[:], in0=v, scalar1=float(hi), scalar2=None,
                                    op0=OP.is_le)
            nc.vector.tensor_tensor(out=r[:], in0=a[:], in1=b2[:], op=OP.mult)
            return r

        vx0 = in_range(x0f[:], 0, 63); vx1 = in_range(x0f[:], -1, 62)
        vy0 = in_range(y0f[:], 0, 63); vy1 = in_range(y0f[:], -1, 62)
        wx1 = f16(); wy1 = f16()
        nc.vector.tensor_tensor(out=wx1[:], in0=xs[:], in1=x0f[:], op=OP.subtract)
        nc.vector.tensor_tensor(out=wy1[:], in0=ys[:], in1=y0f[:], op=OP.subtract)
        wx0m = f16(); wx1m = f16(); wy0m = f16(); wy1m = f16()
        nc.vector.scalar_tensor_tensor(out=wx0m[:], in0=wx1[:], scalar=1.0,
                                       in1=vx0[:], op0=OP.subtract, op1=OP.mult)
        nc.vector.tensor_scalar_mul(wx0m[:], wx0m[:], -1.0)
        nc.vector.tensor_tensor(out=wx1m[:], in0=wx1[:], in1=vx1[:], op=OP.mult)
        nc.vector.scalar_tensor_tensor(out=wy0m[:], in0=wy1[:], scalar=1.0,
                                       in1=vy0[:], op0=OP.subtract, op1=OP.mult)
        nc.vector.tensor_scalar_mul(wy0m[:], wy0m[:], -1.0)
        nc.vector.tensor_tensor(out=wy1m[:], in0=wy1[:], in1=vy1[:], op=OP.mult)
        nc.vector.tensor_tensor(out=Wb[:, 0:16], in0=wy0m[:], in1=wx0m[:], op=OP.mult)
        nc.vector.tensor_tensor(out=Wb[:, 16:32], in0=wy0m[:], in1=wx1m[:], op=OP.mult)
        nc.vector.tensor_tensor(out=Wb[:, 32:48], in0=wy1m[:], in1=wx0m[:], op=OP.mult)
        nc.vector.tensor_tensor(out=Wb[:, 48:64], in0=wy1m[:], in1=wx1m[:], op=OP.mult)
        xm = f16(); ym0 = f16(); ym1 = f16()
        nc.vector.tensor_scalar(out=xm[:], in0=x0f[:], scalar1=-1.0, scalar2=63.0,
                                op0=OP.max, op1=OP.min)
        nc.vector.tensor_scalar(out=ym0[:], in0=y0f[:], scalar1=0.0, scalar2=63.0,
                                op0=OP.max, op1=OP.min)
        nc.vector.tensor_scalar(out=ym1[:], in0=y0f[:], scalar1=1.0, scalar2=0.0,
                                op0=OP.add, op1=OP.max)
        nc.vector.tensor_scalar_min(ym1[:], ym1[:], 63.0)
        IDXf = offp.tile([128, 32], F32)
        nc.vector.scalar_tensor_tensor(out=IDXf[:, 0:16], in0=ym0[:], scalar=64.0,
                                       in1=xm[:], op0=OP.mult, op1=OP.add)
        nc.vector.scalar_tensor_tensor(out=IDXf[:, 16:32], in0=ym1[:], scalar=64.0,
                                       in1=xm[:], op0=OP.mult, op1=OP.add)
        nc.vector.tensor_copy(IDX[:], IDXf[:])

        psOff.release(); offp.release()
        qts_pool.release(); psQT.release(); psA.release()
        if stage < 2:
            warm_pool.release(); x_pool.release(); wq_pool.release()
            drp.release(); q_pool.release(); wkv_pool.release(); P0.release()
            nc.compile(); return nc

        # ---------------- gathers + bilinear + kvf ----------------
        gpool = tc.alloc_tile_pool(name="gp", bufs=3)
        psT = tc.alloc_tile_pool(name="psT", bufs=2, space="PSUM")
        qt_flat = qt_map[:]
        for g in range(G):
            Gt = gpool.tile([128, 512], F32, tag="G")
            for yy in range(2):
                for t in range(2):
                    col = yy * 16 + t * 8 + g
                    nc.gpsimd.indirect_dma_start(
                        out=Gt[:, (yy * 2 + t) * 128:(yy * 2 + t + 1) * 128],
                        out_offset=None, in_=qt_flat,
                        in_offset=bass.IndirectOffsetOnAxis(
                            ap=IDX[:, col:col + 1], axis=0),
                        element_offset=(g * MROWS + 1) * D)
            for t in range(2):
                acc = gpool.tile([128, D], F32, tag="acc")
                m = t * 8 + g
                nc.vector.tensor_scalar(out=acc[:], in0=Gt[:, t * 128:t * 128 + 64],
                                        scalar1=Wb[:, m:m + 1], scalar2=None,
                                        op0=OP.mult)
                for yy, xx in ((0, 1), (1, 0), (1, 1)):
                    blk = (yy * 2 + t) * 128 + xx * 64
                    wcol = (2 * yy + xx) * 16 + m
                    nc.vector.scalar_tensor_tensor(
                        out=acc[:], in0=Gt[:, blk:blk + 64],
                        scalar=Wb[:, wcol:wcol + 1], in1=acc[:],
                        op0=OP.mult, op1=OP.add)
                accb = gpool.tile([128, D], BF16, tag="accb")
                nc.vector.tensor_copy(accb[:], acc[:])
                pt = psT.tile([64, 128], BF16, tag="pt")
                nc.tensor.transpose(pt[:], accb[:], ident_bf[:])
                nc.vector.tensor_copy(
                    kvf[(g % 2) * 64:(g % 2) * 64 + 64, g // 2, t * 128:(t + 1) * 128],
                    pt[:])

        if stage < 3:
            psT.release(); gpool.release()
            warm_pool.release(); x_pool.release(); wq_pool.release()
            drp.release(); q_pool.release(); wkv_pool.release(); P0.release()
            nc.compile(); return nc
        # ---------------- k and vT ----------------
        psKV = tc.alloc_tile_pool(name="psKV", bufs=2, space="PSUM")
        for oc in range(4):
            pk = psKV.tile([128, J], F32, tag="pk")
            for c in range(4):
                nc.tensor.matmul(pk[:], wk_sb[:, c, oc * 128:(oc + 1) * 128],
                                 kvf[:, c, :], start=(c == 0), stop=(c == 3))
            nc.vector.tensor_copy(k_sb[:, oc, :], pk[:])
        nc.vector.memset(vT_sb[:], 1.0)
        for jt in range(2):
            pv = psKV.tile([128, INNER], F32, tag="pv")
            for c in range(4):
                nc.tensor.matmul(pv[:], kvf[:, c, jt * 128:(jt + 1) * 128],
                                 wv_sb[:, c, :], start=(c == 0), stop=(c == 3))
            for h in range(8):
                nc.vector.tensor_copy(vT_sb[:, jt, h * 65:h * 65 + 64],
                                      pv[:, h * 64:(h + 1) * 64])
        i0 = _wi[0]
        vdep(k_sb[:, 0, 0:1])
        vdep(vT_sb[:, 0, 0:1])
        nc.tensor.matmul(warm_t[0:2, 24:26], stg[:, i0:i0 + 2],
                         stg[:, i0:i0 + 2], start=True, stop=True)
        psKV.release(); psT.release(); gpool.release()
        warm_pool.release()
        x_pool.release(); wq_pool.release()
        if stage < 4:
            drp.release(); q_pool.release(); wkv_pool.release(); P0.release()
            nc.compile(); return nc

        # ---------------- fused attention + output projection ----------------
        # PSUM budget (8 banks): psS 2 bufs x [128,2,512]f32 (2 banks each) = 4,
        # psAV 2 x [128,4,65]f32 = 2, psTB 1 x [128,512]bf16 = 1,
        # psF 1 x [128,512]f32 = 1.
        outT_pool = tc.alloc_tile_pool(name="otp", bufs=1)
        outT_sb = outT_pool.tile([128, 4, S], BF16)
        ep = tc.alloc_tile_pool(name="ep", bufs=4)
        psS = tc.alloc_tile_pool(name="psS", bufs=2, space="PSUM")
        psAV = tc.alloc_tile_pool(name="psAV", bufs=2, space="PSUM")
        psTB = tc.alloc_tile_pool(name="psTB", bufs=1, space="PSUM")
        psF = tc.alloc_tile_pool(name="psF", bufs=1, space="PSUM")
        nrm = tc.alloc_tile_pool(name="nrm", bufs=3)
        recp = tc.alloc_tile_pool(name="recp", bufs=4)
        yev = tc.alloc_tile_pool(name="yev", bufs=2)

        E_tiles = {}
        outF_tiles = {}

        def emit_sim(u):
            """sim + exp for unit u = (blk, hp): 1024 cols of E for 2 heads."""
            blk, hp = u // 4, u % 4
            bs = blk * 1024
            E = ep.tile([128, 2, 2, 1024], BF16, tag="E")
            E_tiles[u] = E
            for jt in range(2):
                for half in range(2):
                    cs = bs + half * 512
                    ps = psS.tile([128, 2, 512], F32, tag="ps")
                    for hh in range(2):
                        nc.tensor.matmul(
                            ps[:, hh, :],
                            k_sb[hh * 64:(hh + 1) * 64, hp, jt * 128:(jt + 1) * 128],
                            q_sb[hh * 64:(hh + 1) * 64, hp, cs:cs + 512],
                            start=True, stop=True)
                    nc.scalar.activation(
                        E[:, :, jt, half * 512:(half + 1) * 512], ps[:],
                        AF.Exp, scale=SCALE)

        def emit_av_mm(u):
            """AV matmuls + normalize into outF tiles for unit u."""
            blk, hp = u // 4, u % 4
            E = E_tiles.pop(u)
            for sl in range(2):  # 512-col halves of the block
                outF = nrm.tile([128, 4, 128], BF16, tag="outF")
                outF_tiles[(u, sl)] = outF
                for hh in range(2):
                    h = 2 * hp + hh
                    pav = psAV.tile([128, 4, 65], F32, tag="pav")
                    for ck in range(4):
                        cl = sl * 512 + ck * 128
                        for jt in range(2):
                            nc.tensor.matmul(
                                pav[:, ck, :], E[:, hh, jt, cl:cl + 128],
                                vT_sb[:, jt, h * 65:(h + 1) * 65],
                                start=(jt == 0), stop=(jt == 1))
                    rec = recp.tile([128, 4], F32, tag="rec")
                    nc.vector.reciprocal(rec[:], pav[:, :, 64])
                    nc.vector.tensor_tensor(
                        out=outF[:, :, hh * 64:(hh + 1) * 64],
                        in0=pav[:, :, 0:64],
                        in1=rec[:].unsqueeze(-1).to_broadcast((128, 4, 64)),
                        op=OP.mult)

        def emit_ptb(u):
            """transpose outF into outT columns for unit u."""
            blk, hp = u // 4, u % 4
            for sl in range(2):
                sb2 = blk * 2 + sl
                outF = outF_tiles.pop((u, sl))
                ptb = psTB.tile([128, 512], BF16, tag="ptb")
                for ck in range(4):
                    nc.tensor.transpose(ptb[:, ck * 128:(ck + 1) * 128],
                                        outF[:, ck, :], ident_bf[:])
                nc.vector.tensor_copy(
                    outT_sb[:, hp, sb2 * 512:(sb2 + 1) * 512], ptb[:])

        def emit_final_chunk(sb2, oc):
            pf = psF.tile([128, 512], F32, tag="pf")
            for ic in range(4):
                nc.tensor.matmul(pf[:], wo_sb[:, ic, oc * 128:(oc + 1) * 128],
                                 outT_sb[:, ic, sb2 * 512:(sb2 + 1) * 512],
                                 start=(ic == 0), stop=(ic == 3))
            ye = yev.tile([128, 512], F32, tag="ye")
            nc.vector.tensor_scalar(out=ye[:], in0=pf[:],
                                    scalar1=bout_sb[:, oc:oc + 1], scalar2=None,
                                    op0=OP.add)
            nc.sync.dma_start(
                y_out.ap()[oc * 128:(oc + 1) * 128,
                           sb2 * 512:(sb2 + 1) * 512], ye[:])

        # software pipeline over 16 (blk, hp) units; final-projection chunks
        # of a completed block are drip-fed between later units so the
        # single-buffered psF never stalls the tensor queue.
        pending_final = []
        emit_sim(0)
        emit_sim(1)
        emit_sim(2)
        for u in range(16):
            emit_av_mm(u)
            if u + 3 < 16:
                emit_sim(u + 3)
            emit_ptb(u)
            if u % 4 == 3:
                blk = u // 4
                pending_final.extend(
                    (blk * 2 + sl, oc) for sl in range(2) for oc in range(4))
            for _ in range(2):
                if pending_final and u < 15:
                    emit_final_chunk(*pending_final.pop(0))
        while pending_final:
            emit_final_chunk(*pending_final.pop(0))

        yev.release(); recp.release(); nrm.release()
        psF.release(); psTB.release(); psAV.release(); psS.release(); ep.release()
        outT_pool.release()
        drp.release(); q_pool.release(); wkv_pool.release(); P0.release()
    nc.compile()
    return nc


# ---------------------------------------------------------------------------
# Public entry point: full (unsharded) inputs -> full output.
# Data-parallel over batch: image i runs on NeuronCore i (8 cores).
# ---------------------------------------------------------------------------
_NC_CACHE = {}


def _get_nc():
    if "nc" not in _NC_CACHE:
        _NC_CACHE["nc"] = build()
    return _NC_CACHE["nc"]


def kernel(x, w_q, w_off1, b_off1, w_off2, w_kv, w_out, b_out):
    from concourse.bass_utils import run_bass_kernel_spmd
    x = np.asarray(x, np.float32)
    b = x.shape[0]
    assert x.shape == (8, DIM, H, W), f"unexpected x shape {x.shape}"
    wd = prep_weights(w_q, w_off1, b_off1, w_off2, w_kv, w_out, b_out)
    in_maps = [{"x": np.ascontiguousarray(x[i].reshape(DIM, S)), **wd}
               for i in range(b)]
    nc = _get_nc()
    res = run_bass_kernel_spmd(nc, in_maps, core_ids=list(range(b)))
    out = np.stack([res.results[i]["y"].reshape(DIM, H, W) for i in range(b)])
    return out.astype(np.float32)


# revision 35
# speedup vs baseline: 1.4467x; 1.1704x over previous
import sys
sys.path.insert(0, '/opt/trn_rl_repo')
"""Deformable-attention Bass kernel (one batch image per core).

Pipelined rewrite: single q GEMM (natural layout) + tensor-engine
transposes to build the bf16 gather map, and a fused attention+output
phase that software-pipelines sim -> exp -> AV -> transpose -> final
projection per 1024-column block so the PE array never idles.
"""
import numpy as np
import concourse.bass as bass
import concourse.tile as tile
from concourse import bacc, mybir

F32 = mybir.dt.float32
F32R = mybir.dt.float32r
BF16 = mybir.dt.bfloat16
I32 = mybir.dt.int32
AF = mybir.ActivationFunctionType
OP = mybir.AluOpType

DIM = 512; INNER = 512; H = W = 64; S = H * W
G = 8; D = 64; HEADS = 8; GH = GW = 16; J = GH * GW
SCALE = D ** -0.5
C15 = 64.0 / 15.0
MROWS = S + 2  # per-group map rows incl front/back guard


def host_constants():
    j_of = (np.arange(2)[None, :, None] * 128 + np.arange(128)[:, None, None])
    j_of = np.broadcast_to(j_of, (128, 2, 8)).reshape(128, 16)  # [p, m=t*8+g]
    meshA = (j_of // GW) * C15 - 0.5
    meshB = (j_of % GW) * C15 - 0.5
    return meshA.astype(np.float32), meshB.astype(np.float32)


def prep_weights(w_q, w_off1, b_off1, w_off2, w_kv, w_out, b_out):
    w_q = np.asarray(w_q, np.float32); w_kv = np.asarray(w_kv, np.float32)
    w_out = np.asarray(w_out, np.float32)
    W2 = np.zeros((2, INNER, G), np.float32)
    for g in range(G):
        for k in range(2):
            W2[k, g * D:(g + 1) * D, g] = np.asarray(w_off2, np.float32)[k]
    return {
        "w_qT": np.ascontiguousarray(w_q.T),
        "w_kT": np.ascontiguousarray(w_kv[:INNER].T),
        "w_vT": np.ascontiguousarray(w_kv[INNER:].T),
        "w_oT": np.ascontiguousarray(w_out.T),
        "w1v": np.ascontiguousarray(np.tile(np.asarray(w_off1, np.float32), G))[:, None],
        "b1v": np.ascontiguousarray(np.tile(np.asarray(b_off1, np.float32), G))[:, None],
        "W2x": np.ascontiguousarray(W2[0]),
        "W2y": np.ascontiguousarray(W2[1]),
        "b_out": np.asarray(b_out, np.float32)[:, None],
    }


def build(stage=5):
    nc = bacc.Bacc("TRN2", target_bir_lowering=False)
    x_in = nc.dram_tensor("x", [DIM, S], F32, kind="ExternalInput")
    w_qT = nc.dram_tensor("w_qT", [DIM, INNER], F32, kind="ExternalInput")
    w_kT = nc.dram_tensor("w_kT", [INNER, INNER], F32, kind="ExternalInput")
    w_vT = nc.dram_tensor("w_vT", [INNER, INNER], F32, kind="ExternalInput")
    w_oT = nc.dram_tensor("w_oT", [INNER, DIM], F32, kind="ExternalInput")
    w1v = nc.dram_tensor("w1v", [INNER, 1], F32, kind="ExternalInput")
    b1v = nc.dram_tensor("b1v", [INNER, 1], F32, kind="ExternalInput")
    W2x = nc.dram_tensor("W2x", [INNER, G], F32, kind="ExternalInput")
    W2y = nc.dram_tensor("W2y", [INNER, G], F32, kind="ExternalInput")
    b_out = nc.dram_tensor("b_out", [DIM, 1], F32, kind="ExternalInput")
    y_out = nc.dram_tensor("y", [DIM, S], F32, kind="ExternalOutput")

    meshA_np, meshB_np = host_constants()
    meshA_d = nc.inline_tensor(meshA_np, "meshA")
    meshB_d = nc.inline_tensor(meshB_np, "meshB")

    with tile.TileContext(nc) as tc:
        # ---------------- persistent pool ----------------
        P0 = tc.alloc_tile_pool(name="P0", bufs=1)
        ident = P0.tile([128, 128], F32)
        from concourse.masks import make_identity
        make_identity(nc, ident[:])
        ident_bf = P0.tile([128, 128], BF16)
        nc.vector.tensor_copy(ident_bf[:], ident[:])
        meshA = P0.tile([128, 16], F32); meshB = P0.tile([128, 16], F32)
        nc.sync.dma_start(meshA[:], meshA_d.ap())
        nc.sync.dma_start(meshB[:], meshB_d.ap())
        w1_sb = P0.tile([128, 4], F32); b1_sb = P0.tile([128, 4], F32)
        nc.sync.dma_start(w1_sb[:], w1v.ap().rearrange("(c p) one -> p (c one)", p=128))
        nc.sync.dma_start(b1_sb[:], b1v.ap().rearrange("(c p) one -> p (c one)", p=128))
        W2x_sb = P0.tile([128, 4, G], F32); W2y_sb = P0.tile([128, 4, G], F32)
        nc.sync.dma_start(W2x_sb[:], W2x.ap().rearrange("(c p) g -> p c g", p=128))
        nc.sync.dma_start(W2y_sb[:], W2y.ap().rearrange("(c p) g -> p c g", p=128))
        bout_sb = P0.tile([128, 4], F32)
        nc.sync.dma_start(bout_sb[:], b_out.ap().rearrange("(c p) one -> p (c one)", p=128))
        IDX = P0.tile([128, 32], I32)
        Wb = P0.tile([128, 64], F32)
        kvf = P0.tile([128, 4, J], BF16)
        k_sb = P0.tile([128, 4, J], BF16)
        vT_sb = P0.tile([128, 2, 8 * 65], BF16)
        wo_sb = P0.tile([128, 4, DIM], BF16)
        wkv_pool = tc.alloc_tile_pool(name="wkvp", bufs=1)
        wk_sb = wkv_pool.tile([128, 4, INNER], BF16)
        wv_sb = wkv_pool.tile([128, 4, INNER], BF16)

        q_pool = tc.alloc_tile_pool(name="qp", bufs=1)
        q_sb = q_pool.tile([128, 4, S], BF16)

        # DRAM scratch: per-group transposed-q map (bf16) for the gathers
        drp = tc.alloc_tile_pool(name="dr", bufs=1, space="DRAM")
        qt_map = drp.tile([G * MROWS, D], F32)
        zt = P0.tile([G, 2, D], F32)
        nc.vector.memset(zt[:], 0.0)
        guard_dst = bass.AP(tensor=qt_map[:].tensor, offset=qt_map[:].offset,
                            ap=[[MROWS * D, G], [(MROWS - 1) * D, 2], [1, D]])
        nc.sync.dma_start(guard_dst, zt[:])

        # ---------------- phase A: x load + natural-q GEMM + qT ----------------
        wq_pool = tc.alloc_tile_pool(name="wqp", bufs=1)
        wq_sb = wq_pool.tile([128, 4, INNER], F32R)
        for c in range(4):
            nc.gpsimd.dma_start(wq_sb[:, c, :], w_qT.ap()[c * 128:(c + 1) * 128, :])
        x_pool = tc.alloc_tile_pool(name="xp", bufs=1)
        x_sb = x_pool.tile([128, 4, S], F32R)
        for q4 in range(4):
            for c in range(4):
                nc.gpsimd.dma_start(
                    x_sb[:, c, q4 * 1024:(q4 + 1) * 1024],
                    x_in.ap()[c * 128:(c + 1) * 128, q4 * 1024:(q4 + 1) * 1024])

        for c in range(4):
            nc.gpsimd.dma_start(wk_sb[:, c, :], w_kT.ap()[c * 128:(c + 1) * 128, :])
            nc.gpsimd.dma_start(wv_sb[:, c, :], w_vT.ap()[c * 128:(c + 1) * 128, :])
            nc.gpsimd.dma_start(wo_sb[:, c, :], w_oT.ap()[c * 128:(c + 1) * 128, :])

        # Dependency absorption: a matmul's LDWEIGHTS slot allows only ONE
        # sync wait, so a matmul whose operands arrive via two different DMA
        # queues fails walrus codegen. Funnel every weight-DMA dependency
        # through tiny vector copies into a bf16 staging tile, then order the
        # tensor queue after all of them with a single warm matmul.
        warm_pool = tc.alloc_tile_pool(name="warm", bufs=1, space="PSUM")
        warm_t = warm_pool.tile([32, 32], F32)
        stg = P0.tile([128, 32], BF16)
        _wi = [0]

        def vdep(ap2):
            if ap2.dtype == F32R:
                ap2 = ap2.bitcast(F32)
            nc.vector.tensor_copy(stg[:, _wi[0]:_wi[0] + 1], ap2)
            _wi[0] += 1

        for c in range(4):
            vdep(wq_sb[:, c, 0:1])
        vdep(ident[:, 0:1])
        vdep(ident_bf[:, 0:1])
        nc.tensor.matmul(warm_t[0:_wi[0], 0:_wi[0]], stg[:, 0:_wi[0]],
                         stg[:, 0:_wi[0]], start=True, stop=True)

        psA = tc.alloc_tile_pool(name="psA", bufs=3, space="PSUM")
        psQT = tc.alloc_tile_pool(name="psQT", bufs=2, space="PSUM")
        qts_pool = tc.alloc_tile_pool(name="qts", bufs=3)
        for sb in range(8):
            s0 = sb * 512
            for oc in range(4):
                pq = psA.tile([128, 512], F32, tag="pq")
                for c in range(4):
                    nc.tensor.matmul(pq[:], wq_sb[:, c, oc * 128:(oc + 1) * 128],
                                     x_sb[:, c, s0:s0 + 512],
                                     start=(c == 0), stop=(c == 3))
                nc.vector.tensor_copy(q_sb[:, oc, s0:s0 + 512], pq[:])
            # transpose this 512-col block into qt_map rows (4 chunks of 128 s)
            for ch in range(4):
                cs = s0 + ch * 128
                ptq = psQT.tile([128, 512], BF16, tag="ptq")
                for oc in range(4):
                    nc.tensor.transpose(ptq[:, oc * 128:(oc + 1) * 128],
                                        q_sb[:, oc, cs:cs + 128], ident_bf[:])
                qts = qts_pool.tile([128, 512], F32, tag="qts")
                nc.vector.tensor_copy(qts[:], ptq[:])
                dst = bass.AP(tensor=qt_map[:].tensor,
                              offset=qt_map[:].offset + (1 + cs) * D,
                              ap=[[D, 128], [MROWS * D, G], [1, D]])
                nc.sync.dma_start(dst, qts[:].rearrange("p (g d) -> p g d", g=G))

        # ---------------- offsets ----------------
        offp = tc.alloc_tile_pool(name="offp", bufs=1)
        psOff = tc.alloc_tile_pool(name="psOff", bufs=1, space="PSUM")
        t_sb = offp.tile([128, 4, J], F32)
        for ic in range(4):
            pqd = psOff.tile([128, J], F32, tag="pqd")
            for c in range(4):
                base = x_sb[:, c, :]
                rhs = bass.AP(tensor=base.tensor, offset=base.offset,
                              ap=[list(base.ap[0]), [256, 16], [4, 16]])
                nc.tensor.matmul(pqd[:], wq_sb[:, c, ic * 128:(ic + 1) * 128], rhs,
                                 start=(c == 0), stop=(c == 3))
            nc.scalar.activation(t_sb[:, ic, :], pqd[:], AF.Gelu,
                                 bias=b1_sb[:, ic:ic + 1], scale=w1_sb[:, ic:ic + 1])
        offx = offp.tile([128, 16], F32); offy = offp.tile([128, 16], F32)
        for jt in range(2):
            pxy = psOff.tile([128, 2, G], F32, tag="pxy")
            px = pxy[:, 0, :]; py = pxy[:, 1, :]
            for c in range(4):
                nc.tensor.matmul(px, t_sb[:, c, jt * 128:(jt + 1) * 128],
                                 W2x_sb[:, c, :], start=(c == 0), stop=(c == 3))
            for c in range(4):
                nc.tensor.matmul(py, t_sb[:, c, jt * 128:(jt + 1) * 128],
                                 W2y_sb[:, c, :], start=(c == 0), stop=(c == 3))
            nc.scalar.activation(offx[:, jt * 8:(jt + 1) * 8], px, AF.Tanh)
            nc.scalar.activation(offy[:, jt * 8:(jt + 1) * 8], py, AF.Tanh)

        _fc = [0]
        def f16():
            _fc[0] += 1
            return offp.tile([128, 16], F32, name=f"f16_{_fc[0]}", tag=f"f16_{_fc[0]}")

        xs = f16(); ys = f16()
        nc.vector.scalar_tensor_tensor(out=xs[:], in0=offx[:], scalar=4.0 * C15,
                                       in1=meshA[:], op0=OP.mult, op1=OP.add)
        nc.vector.scalar_tensor_tensor(out=ys[:], in0=offy[:], scalar=4.0 * C15,
                                       in1=meshB[:], op0=OP.mult, op1=OP.add)

        def floor_of(src):
            _fc[0] += 1
            ti = offp.tile([128, 16], I32, name=f"i16_{_fc[0]}", tag=f"i16_{_fc[0]}")
            nc.vector.tensor_copy(ti[:], src)
            tf = f16()
            nc.vector.tensor_copy(tf[:], ti[:])
            gt = f16()
            nc.vector.tensor_tensor(out=gt[:], in0=tf[:], in1=src, op=OP.is_gt)
            fl = f16()
            nc.vector.tensor_tensor(out=fl[:], in0=tf[:], in1=gt[:], op=OP.subtract)
            return fl

        x0f = floor_of(xs[:]); y0f = floor_of(ys[:])

        def in_range(v, lo, hi):
            a = f16(); b2 = f16(); r = f16()
            nc.vector.tensor_scalar(out=a[:], in0=v, scalar1=float(lo), scalar2=None,
                                    op0=OP.is_ge)
            nc.vector.tensor_scalar(out=b2[:], in0=v, scalar1=float(hi), scalar2=None,
                                    op0=OP.is_le)
            nc.vector.tensor_tensor(out=r[:], in0=a[:], in1=b2[:], op=OP.mult)
            return r

        vx0 = in_range(x0f[:], 0, 63); vx1 = in_range(x0f[:], -1, 62)
        vy0 = in_range(y0f[:], 0, 63); vy1 = in_range(y0f[:], -1, 62)
        wx1 = f16(); wy1 = f16()
        nc.vector.tensor_tensor(out=wx1[:], in0=xs[:], in1=x0f[:], op=OP.subtract)
        nc.vector.tensor_tensor(out=wy1[:], in0=ys[:], in1=y0f[:], op=OP.subtract)
        wx0m = f16(); wx1m = f16(); wy0m = f16(); wy1m = f16()
        nc.vector.scalar_tensor_tensor(out=wx0m[:], in0=wx1[:], scalar=1.0,
                                       in1=vx0[:], op0=OP.subtract, op1=OP.mult)
        nc.vector.tensor_scalar_mul(wx0m[:], wx0m[:], -1.0)
        nc.vector.tensor_tensor(out=wx1m[:], in0=wx1[:], in1=vx1[:], op=OP.mult)
        nc.vector.scalar_tensor_tensor(out=wy0m[:], in0=wy1[:], scalar=1.0,
                                       in1=vy0[:], op0=OP.subtract, op1=OP.mult)
        nc.vector.tensor_scalar_mul(wy0m[:], wy0m[:], -1.0)
        nc.vector.tensor_tensor(out=wy1m[:], in0=wy1[:], in1=vy1[:], op=OP.mult)
        nc.vector.tensor_tensor(out=Wb[:, 0:16], in0=wy0m[:], in1=wx0m[:], op=OP.mult)
        nc.vector.tensor_tensor(out=Wb[:, 16:32], in0=wy0m[:], in1=wx1m[:], op=OP.mult)
        nc.vector.tensor_tensor(out=Wb[:, 32:48], in0=wy1m[:], in1=wx0m[:], op=OP.mult)
        nc.vector.tensor_tensor(out=Wb[:, 48:64], in0=wy1m[:], in1=wx1m[:], op=OP.mult)
        xm = f16(); ym0 = f16(); ym1 = f16()
        nc.vector.tensor_scalar(out=xm[:], in0=x0f[:], scalar1=-1.0, scalar2=63.0,
                                op0=OP.max, op1=OP.min)
        nc.vector.tensor_scalar(out=ym0[:], in0=y0f[:], scalar1=0.0, scalar2=63.0,
                                op0=OP.max, op1=OP.min)
        nc.vector.tensor_scalar(out=ym1[:], in0=y0f[:], scalar1=1.0, scalar2=0.0,
                                op0=OP.add, op1=OP.max)
        nc.vector.tensor_scalar_min(ym1[:], ym1[:], 63.0)
        IDXf = offp.tile([128, 32], F32)
        nc.vector.scalar_tensor_tensor(out=IDXf[:, 0:16], in0=ym0[:], scalar=64.0,
                                       in1=xm[:], op0=OP.mult, op1=OP.add)
        nc.vector.scalar_tensor_tensor(out=IDXf[:, 16:32], in0=ym1[:], scalar=64.0,
                                       in1=xm[:], op0=OP.mult, op1=OP.add)
        nc.vector.tensor_copy(IDX[:], IDXf[:])

        psOff.release(); offp.release()
        qts_pool.release(); psQT.release(); psA.release()
        if stage < 2:
            warm_pool.release(); x_pool.release(); wq_pool.release()
            drp.release(); q_pool.release(); wkv_pool.release(); P0.release()
            nc.compile(); return nc

        # ---------------- gathers + bilinear + kvf ----------------
        gpool = tc.alloc_tile_pool(name="gp", bufs=3)
        psT = tc.alloc_tile_pool(name="psT", bufs=2, space="PSUM")
        qt_flat = qt_map[:]
        for g in range(G):
            Gt = gpool.tile([128, 512], F32, tag="G")
            for yy in range(2):
                for t in range(2):
                    col = yy * 16 + t * 8 + g
                    nc.gpsimd.indirect_dma_start(
                        out=Gt[:, (yy * 2 + t) * 128:(yy * 2 + t + 1) * 128],
                        out_offset=None, in_=qt_flat,
                        in_offset=bass.IndirectOffsetOnAxis(
                            ap=IDX[:, col:col + 1], axis=0),
                        element_offset=(g * MROWS + 1) * D)
            for t in range(2):
                acc = gpool.tile([128, D], F32, tag="acc")
                m = t * 8 + g
                nc.vector.tensor_scalar(out=acc[:], in0=Gt[:, t * 128:t * 128 + 64],
                                        scalar1=Wb[:, m:m + 1], scalar2=None,
                                        op0=OP.mult)
                for yy, xx in ((0, 1), (1, 0), (1, 1)):
                    blk = (yy * 2 + t) * 128 + xx * 64
                    wcol = (2 * yy + xx) * 16 + m
                    nc.vector.scalar_tensor_tensor(
                        out=acc[:], in0=Gt[:, blk:blk + 64],
                        scalar=Wb[:, wcol:wcol + 1], in1=acc[:],
                        op0=OP.mult, op1=OP.add)
                accb = gpool.tile([128, D], BF16, tag="accb")
                nc.vector.tensor_copy(accb[:], acc[:])
                pt = psT.tile([64, 128], BF16, tag="pt")
                nc.tensor.transpose(pt[:], accb[:], ident_bf[:])
                nc.vector.tensor_copy(
                    kvf[(g % 2) * 64:(g % 2) * 64 + 64, g // 2, t * 128:(t + 1) * 128],
                    pt[:])

        if stage < 3:
            psT.release(); gpool.release()
            warm_pool.release(); x_pool.release(); wq_pool.release()
            drp.release(); q_pool.release(); wkv_pool.release(); P0.release()
            nc.compile(); return nc
        # ---------------- k and vT ----------------
        i1 = _wi[0]
        for c in range(4):
            vdep(wk_sb[:, c, 0:1])
            vdep(wv_sb[:, c, 0:1])
            vdep(wo_sb[:, c, 0:1])
        nc.tensor.matmul(warm_t[0:12, 8:20], stg[:, i1:i1 + 12],
                         stg[:, i1:i1 + 12], start=True, stop=True)
        psKV = tc.alloc_tile_pool(name="psKV", bufs=2, space="PSUM")
        for oc in range(4):
            pk = psKV.tile([128, J], F32, tag="pk")
            for c in range(4):
                nc.tensor.matmul(pk[:], wk_sb[:, c, oc * 128:(oc + 1) * 128],
                                 kvf[:, c, :], start=(c == 0), stop=(c == 3))
            nc.vector.tensor_copy(k_sb[:, oc, :], pk[:])
        nc.vector.memset(vT_sb[:], 1.0)
        for jt in range(2):
            pv = psKV.tile([128, INNER], F32, tag="pv")
            for c in range(4):
                nc.tensor.matmul(pv[:], kvf[:, c, jt * 128:(jt + 1) * 128],
                                 wv_sb[:, c, :], start=(c == 0), stop=(c == 3))
            for h in range(8):
                nc.vector.tensor_copy(vT_sb[:, jt, h * 65:h * 65 + 64],
                                      pv[:, h * 64:(h + 1) * 64])
        i0 = _wi[0]
        vdep(k_sb[:, 0, 0:1])
        vdep(vT_sb[:, 0, 0:1])
        nc.tensor.matmul(warm_t[0:2, 24:26], stg[:, i0:i0 + 2],
                         stg[:, i0:i0 + 2], start=True, stop=True)
        psKV.release(); psT.release(); gpool.release()
        warm_pool.release()
        x_pool.release(); wq_pool.release()
        if stage < 4:
            drp.release(); q_pool.release(); wkv_pool.release(); P0.release()
            nc.compile(); return nc

        # ---------------- fused attention + output projection ----------------
        # PSUM budget (8 banks): psS 2 bufs x [128,2,512]f32 (2 banks each) = 4,
        # psAV 2 x [128,4,65]f32 = 2, psTB 1 x [128,512]bf16 = 1,
        # psF 1 x [128,512]f32 = 1.
        outT_pool = tc.alloc_tile_pool(name="otp", bufs=1)
        outT_sb = outT_pool.tile([128, 4, S], BF16)
        ep = tc.alloc_tile_pool(name="ep", bufs=3)
        psS = tc.alloc_tile_pool(name="psS", bufs=2, space="PSUM")
        psAV = tc.alloc_tile_pool(name="psAV", bufs=2, space="PSUM")
        psTB = tc.alloc_tile_pool(name="psTB", bufs=1, space="PSUM")
        psF = tc.alloc_tile_pool(name="psF", bufs=1, space="PSUM")
        nrm = tc.alloc_tile_pool(name="nrm", bufs=3)
        recp = tc.alloc_tile_pool(name="recp", bufs=4)
        yev = tc.alloc_tile_pool(name="yev", bufs=2)

        E_tiles = {}
        outF_tiles = {}

        def emit_sim(u):
            """sim + exp for unit u = (blk, hp): 1024 cols of E for 2 heads."""
            blk, hp = u // 4, u % 4
            bs = blk * 1024
            E = ep.tile([128, 2, 2, 1024], BF16, tag="E")
            E_tiles[u] = E
            for jt in range(2):
                for half in range(2):
                    cs = bs + half * 512
                    ps = psS.tile([128, 2, 512], F32, tag="ps")
                    for hh in range(2):
                        nc.tensor.matmul(
                            ps[:, hh, :],
                            k_sb[hh * 64:(hh + 1) * 64, hp, jt * 128:(jt + 1) * 128],
                            q_sb[hh * 64:(hh + 1) * 64, hp, cs:cs + 512],
                            start=True, stop=True)
                    nc.scalar.activation(
                        E[:, :, jt, half * 512:(half + 1) * 512], ps[:],
                        AF.Exp, scale=SCALE)

        def emit_av_mm(u):
            """AV matmuls + normalize into outF tiles for unit u."""
            blk, hp = u // 4, u % 4
            E = E_tiles.pop(u)
            for sl in range(2):  # 512-col halves of the block
                outF = nrm.tile([128, 4, 128], BF16, tag="outF")
                outF_tiles[(u, sl)] = outF
                for hh in range(2):
                    h = 2 * hp + hh
                    pav = psAV.tile([128, 4, 65], F32, tag="pav")
                    for ck in range(4):
                        cl = sl * 512 + ck * 128
                        for jt in range(2):
                            nc.tensor.matmul(
                                pav[:, ck, :], E[:, hh, jt, cl:cl + 128],
                                vT_sb[:, jt, h * 65:(h + 1) * 65],
                                start=(jt == 0), stop=(jt == 1))
                    rec = recp.tile([128, 4], F32, tag="rec")
                    nc.vector.reciprocal(rec[:], pav[:, :, 64])
                    nc.vector.tensor_tensor(
                        out=outF[:, :, hh * 64:(hh + 1) * 64],
                        in0=pav[:, :, 0:64],
                        in1=rec[:].unsqueeze(-1).to_broadcast((128, 4, 64)),
                        op=OP.mult)

        def emit_ptb(u):
            """transpose outF into outT columns for unit u."""
            blk, hp = u // 4, u % 4
            for sl in range(2):
                sb2 = blk * 2 + sl
                outF = outF_tiles.pop((u, sl))
                ptb = psTB.tile([128, 512], BF16, tag="ptb")
                for ck in range(4):
                    nc.tensor.transpose(ptb[:, ck * 128:(ck + 1) * 128],
                                        outF[:, ck, :], ident_bf[:])
                nc.vector.tensor_copy(
                    outT_sb[:, hp, sb2 * 512:(sb2 + 1) * 512], ptb[:])

        def emit_final_chunk(sb2, oc):
            pf = psF.tile([128, 512], F32, tag="pf")
            for ic in range(4):
                nc.tensor.matmul(pf[:], wo_sb[:, ic, oc * 128:(oc + 1) * 128],
                                 outT_sb[:, ic, sb2 * 512:(sb2 + 1) * 512],
                                 start=(ic == 0), stop=(ic == 3))
            ye = yev.tile([128, 512], F32, tag="ye")
            nc.vector.tensor_scalar(out=ye[:], in0=pf[:],
                                    scalar1=bout_sb[:, oc:oc + 1], scalar2=None,
                                    op0=OP.add)
            nc.sync.dma_start(
                y_out.ap()[oc * 128:(oc + 1) * 128,
                           sb2 * 512:(sb2 + 1) * 512], ye[:])

        # software pipeline over 16 (blk, hp) units; final-projection chunks
        # of a completed block are drip-fed between later units so the
        # single-buffered psF never stalls the tensor queue.
        pending_final = []
        emit_sim(0)
        emit_sim(1)
        for u in range(16):
            emit_av_mm(u)
            if u + 2 < 16:
                emit_sim(u + 2)
            emit_ptb(u)
            if u % 4 == 3:
                blk = u // 4
                pending_final.extend(
                    (blk * 2 + sl, oc) for sl in range(2) for oc in range(4))
            for _ in range(2):
                if pending_final and u < 15:
                    emit_final_chunk(*pending_final.pop(0))
        while pending_final:
            emit_final_chunk(*pending_final.pop(0))

        yev.release(); recp.release(); nrm.release()
        psF.release(); psTB.release(); psAV.release(); psS.release(); ep.release()
        outT_pool.release()
        drp.release(); q_pool.release(); wkv_pool.release(); P0.release()
    nc.compile()
    return nc


# ---------------------------------------------------------------------------
# Public entry point: full (unsharded) inputs -> full output.
# Data-parallel over batch: image i runs on NeuronCore i (8 cores).
# ---------------------------------------------------------------------------
_NC_CACHE = {}


def _get_nc():
    if "nc" not in _NC_CACHE:
        _NC_CACHE["nc"] = build()
    return _NC_CACHE["nc"]


def kernel(x, w_q, w_off1, b_off1, w_off2, w_kv, w_out, b_out):
    from concourse.bass_utils import run_bass_kernel_spmd
    x = np.asarray(x, np.float32)
    b = x.shape[0]
    assert x.shape == (8, DIM, H, W), f"unexpected x shape {x.shape}"
    wd = prep_weights(w_q, w_off1, b_off1, w_off2, w_kv, w_out, b_out)
    in_maps = [{"x": np.ascontiguousarray(x[i].reshape(DIM, S)), **wd}
               for i in range(b)]
    nc = _get_nc()
    res = run_bass_kernel_spmd(nc, in_maps, core_ids=list(range(b)))
    out = np.stack([res.results[i]["y"].reshape(DIM, H, W) for i in range(b)])
    return out.astype(np.float32)


# revision 37
# speedup vs baseline: 1.4694x; 1.0157x over previous
import sys
sys.path.insert(0, '/opt/trn_rl_repo')
"""Deformable-attention Bass kernel (one batch image per core).

Pipelined rewrite: single q GEMM (natural layout) + tensor-engine
transposes to build the bf16 gather map, and a fused attention+output
phase that software-pipelines sim -> exp -> AV -> transpose -> final
projection per 1024-column block so the PE array never idles.
"""
import numpy as np
import concourse.bass as bass
import concourse.tile as tile
from concourse import bacc, mybir

F32 = mybir.dt.float32
F32R = mybir.dt.float32r
BF16 = mybir.dt.bfloat16
I32 = mybir.dt.int32
AF = mybir.ActivationFunctionType
OP = mybir.AluOpType

DIM = 512; INNER = 512; H = W = 64; S = H * W
G = 8; D = 64; HEADS = 8; GH = GW = 16; J = GH * GW
SCALE = D ** -0.5
C15 = 64.0 / 15.0
MROWS = S + 2  # per-group map rows incl front/back guard


def host_constants():
    j_of = (np.arange(2)[None, :, None] * 128 + np.arange(128)[:, None, None])
    j_of = np.broadcast_to(j_of, (128, 2, 8)).reshape(128, 16)  # [p, m=t*8+g]
    meshA = (j_of // GW) * C15 - 0.5
    meshB = (j_of % GW) * C15 - 0.5
    return meshA.astype(np.float32), meshB.astype(np.float32)


def prep_weights(w_q, w_off1, b_off1, w_off2, w_kv, w_out, b_out):
    w_q = np.asarray(w_q, np.float32); w_kv = np.asarray(w_kv, np.float32)
    w_out = np.asarray(w_out, np.float32)
    W2 = np.zeros((2, INNER, G), np.float32)
    for g in range(G):
        for k in range(2):
            W2[k, g * D:(g + 1) * D, g] = np.asarray(w_off2, np.float32)[k]
    return {
        "w_qT": np.ascontiguousarray(w_q.T),
        "w_kT": np.ascontiguousarray(w_kv[:INNER].T),
        "w_vT": np.ascontiguousarray(w_kv[INNER:].T),
        "w_oT": np.ascontiguousarray(w_out.T),
        "w1v": np.ascontiguousarray(np.tile(np.asarray(w_off1, np.float32), G))[:, None],
        "b1v": np.ascontiguousarray(np.tile(np.asarray(b_off1, np.float32), G))[:, None],
        "W2x": np.ascontiguousarray(W2[0]),
        "W2y": np.ascontiguousarray(W2[1]),
        "b_out": np.asarray(b_out, np.float32)[:, None],
    }


def build(stage=5):
    nc = bacc.Bacc("TRN2", target_bir_lowering=False)
    x_in = nc.dram_tensor("x", [DIM, S], F32, kind="ExternalInput")
    w_qT = nc.dram_tensor("w_qT", [DIM, INNER], F32, kind="ExternalInput")
    w_kT = nc.dram_tensor("w_kT", [INNER, INNER], F32, kind="ExternalInput")
    w_vT = nc.dram_tensor("w_vT", [INNER, INNER], F32, kind="ExternalInput")
    w_oT = nc.dram_tensor("w_oT", [INNER, DIM], F32, kind="ExternalInput")
    w1v = nc.dram_tensor("w1v", [INNER, 1], F32, kind="ExternalInput")
    b1v = nc.dram_tensor("b1v", [INNER, 1], F32, kind="ExternalInput")
    W2x = nc.dram_tensor("W2x", [INNER, G], F32, kind="ExternalInput")
    W2y = nc.dram_tensor("W2y", [INNER, G], F32, kind="ExternalInput")
    b_out = nc.dram_tensor("b_out", [DIM, 1], F32, kind="ExternalInput")
    y_out = nc.dram_tensor("y", [DIM, S], F32, kind="ExternalOutput")

    meshA_np, meshB_np = host_constants()
    meshA_d = nc.inline_tensor(meshA_np, "meshA")
    meshB_d = nc.inline_tensor(meshB_np, "meshB")

    with tile.TileContext(nc) as tc:
        # ---------------- persistent pool ----------------
        P0 = tc.alloc_tile_pool(name="P0", bufs=1)
        ident = P0.tile([128, 128], F32)
        from concourse.masks import make_identity
        make_identity(nc, ident[:])
        ident_bf = P0.tile([128, 128], BF16)
        nc.vector.tensor_copy(ident_bf[:], ident[:])
        meshA = P0.tile([128, 16], F32); meshB = P0.tile([128, 16], F32)
        nc.sync.dma_start(meshA[:], meshA_d.ap())
        nc.sync.dma_start(meshB[:], meshB_d.ap())
        w1_sb = P0.tile([128, 4], F32); b1_sb = P0.tile([128, 4], F32)
        nc.sync.dma_start(w1_sb[:], w1v.ap().rearrange("(c p) one -> p (c one)", p=128))
        nc.sync.dma_start(b1_sb[:], b1v.ap().rearrange("(c p) one -> p (c one)", p=128))
        W2x_sb = P0.tile([128, 4, G], F32); W2y_sb = P0.tile([128, 4, G], F32)
        nc.sync.dma_start(W2x_sb[:], W2x.ap().rearrange("(c p) g -> p c g", p=128))
        nc.sync.dma_start(W2y_sb[:], W2y.ap().rearrange("(c p) g -> p c g", p=128))
        bout_sb = P0.tile([128, 4], F32)
        nc.sync.dma_start(bout_sb[:], b_out.ap().rearrange("(c p) one -> p (c one)", p=128))
        IDX = P0.tile([128, 32], I32)
        Wb = P0.tile([128, 64], F32)
        kvf = P0.tile([128, 4, J], BF16)
        k_sb = P0.tile([128, 4, J], BF16)
        vT_sb = P0.tile([128, 2, 8 * 65], BF16)
        wo_sb = P0.tile([128, 4, DIM], BF16)
        for c in range(4):
            nc.gpsimd.dma_start(wo_sb[:, c, :], w_oT.ap()[c * 128:(c + 1) * 128, :])
        wkv_pool = tc.alloc_tile_pool(name="wkvp", bufs=1)
        wk_sb = wkv_pool.tile([128, 4, INNER], BF16)
        wv_sb = wkv_pool.tile([128, 4, INNER], BF16)
        for c in range(4):
            nc.gpsimd.dma_start(wk_sb[:, c, :], w_kT.ap()[c * 128:(c + 1) * 128, :])
            nc.gpsimd.dma_start(wv_sb[:, c, :], w_vT.ap()[c * 128:(c + 1) * 128, :])

        q_pool = tc.alloc_tile_pool(name="qp", bufs=1)
        q_sb = q_pool.tile([128, 4, S], BF16)

        # DRAM scratch: per-group transposed-q map (bf16) for the gathers
        drp = tc.alloc_tile_pool(name="dr", bufs=1, space="DRAM")
        qt_map = drp.tile([G * MROWS, D], F32)
        zt = P0.tile([G, 2, D], F32)
        nc.vector.memset(zt[:], 0.0)
        guard_dst = bass.AP(tensor=qt_map[:].tensor, offset=qt_map[:].offset,
                            ap=[[MROWS * D, G], [(MROWS - 1) * D, 2], [1, D]])
        nc.sync.dma_start(guard_dst, zt[:])

        # ---------------- phase A: x load + natural-q GEMM + qT ----------------
        wq_pool = tc.alloc_tile_pool(name="wqp", bufs=1)
        wq_sb = wq_pool.tile([128, 4, INNER], F32R)
        for c in range(4):
            nc.gpsimd.dma_start(wq_sb[:, c, :], w_qT.ap()[c * 128:(c + 1) * 128, :])
        x_pool = tc.alloc_tile_pool(name="xp", bufs=1)
        x_sb = x_pool.tile([128, 4, S], F32R)
        for q4 in range(4):
            for c in range(4):
                nc.gpsimd.dma_start(
                    x_sb[:, c, q4 * 1024:(q4 + 1) * 1024],
                    x_in.ap()[c * 128:(c + 1) * 128, q4 * 1024:(q4 + 1) * 1024])

        # Dependency absorption: a matmul's LDWEIGHTS slot allows only ONE
        # sync wait, so a matmul whose operands arrive via two different DMA
        # queues fails walrus codegen. Funnel every weight-DMA dependency
        # through tiny vector copies into a bf16 staging tile, then order the
        # tensor queue after all of them with a single warm matmul.
        warm_pool = tc.alloc_tile_pool(name="warm", bufs=1, space="PSUM")
        warm_t = warm_pool.tile([32, 32], F32)
        stg = P0.tile([128, 32], BF16)
        _wi = [0]

        def vdep(ap2):
            if ap2.dtype == F32R:
                ap2 = ap2.bitcast(F32)
            nc.vector.tensor_copy(stg[:, _wi[0]:_wi[0] + 1], ap2)
            _wi[0] += 1

        for c in range(4):
            vdep(wq_sb[:, c, 0:1])
            vdep(wk_sb[:, c, 0:1])
            vdep(wv_sb[:, c, 0:1])
            vdep(wo_sb[:, c, 0:1])
        vdep(ident[:, 0:1])
        vdep(ident_bf[:, 0:1])
        nc.tensor.matmul(warm_t[0:_wi[0], 0:_wi[0]], stg[:, 0:_wi[0]],
                         stg[:, 0:_wi[0]], start=True, stop=True)

        psA = tc.alloc_tile_pool(name="psA", bufs=3, space="PSUM")
        psQT = tc.alloc_tile_pool(name="psQT", bufs=2, space="PSUM")
        qts_pool = tc.alloc_tile_pool(name="qts", bufs=6)
        for sb in range(8):
            s0 = sb * 512
            for oc in range(4):
                pq = psA.tile([128, 512], F32, tag="pq")
                for c in range(4):
                    nc.tensor.matmul(pq[:], wq_sb[:, c, oc * 128:(oc + 1) * 128],
                                     x_sb[:, c, s0:s0 + 512],
                                     start=(c == 0), stop=(c == 3))
                nc.vector.tensor_copy(q_sb[:, oc, s0:s0 + 512], pq[:])
            # transpose this 512-col block into qt_map rows (4 chunks of 128 s)
            for ch in range(4):
                cs = s0 + ch * 128
                ptq = psQT.tile([128, 512], BF16, tag="ptq")
                for oc in range(4):
                    nc.tensor.transpose(ptq[:, oc * 128:(oc + 1) * 128],
                                        q_sb[:, oc, cs:cs + 128], ident_bf[:])
                qts = qts_pool.tile([128, 512], F32, tag="qts")
                nc.vector.tensor_copy(qts[:], ptq[:])
                dst = bass.AP(tensor=qt_map[:].tensor,
                              offset=qt_map[:].offset + (1 + cs) * D,
                              ap=[[D, 128], [MROWS * D, G], [1, D]])
                nc.sync.dma_start(dst, qts[:].rearrange("p (g d) -> p g d", g=G))

        # ---------------- offsets ----------------
        offp = tc.alloc_tile_pool(name="offp", bufs=1)
        psOff = tc.alloc_tile_pool(name="psOff", bufs=1, space="PSUM")
        t_sb = offp.tile([128, 4, J], F32)
        for ic in range(4):
            pqd = psOff.tile([128, J], F32, tag="pqd")
            for c in range(4):
                base = x_sb[:, c, :]
                rhs = bass.AP(tensor=base.tensor, offset=base.offset,
                              ap=[list(base.ap[0]), [256, 16], [4, 16]])
                nc.tensor.matmul(pqd[:], wq_sb[:, c, ic * 128:(ic + 1) * 128], rhs,
                                 start=(c == 0), stop=(c == 3))
            nc.scalar.activation(t_sb[:, ic, :], pqd[:], AF.Gelu,
                                 bias=b1_sb[:, ic:ic + 1], scale=w1_sb[:, ic:ic + 1])
        offx = offp.tile([128, 16], F32); offy = offp.tile([128, 16], F32)
        for jt in range(2):
            pxy = psOff.tile([128, 2, G], F32, tag="pxy")
            px = pxy[:, 0, :]; py = pxy[:, 1, :]
            for c in range(4):
                nc.tensor.matmul(px, t_sb[:, c, jt * 128:(jt + 1) * 128],
                                 W2x_sb[:, c, :], start=(c == 0), stop=(c == 3))
            for c in range(4):
                nc.tensor.matmul(py, t_sb[:, c, jt * 128:(jt + 1) * 128],
                                 W2y_sb[:, c, :], start=(c == 0), stop=(c == 3))
            nc.scalar.activation(offx[:, jt * 8:(jt + 1) * 8], px, AF.Tanh)
            nc.scalar.activation(offy[:, jt * 8:(jt + 1) * 8], py, AF.Tanh)

        _fc = [0]
        def f16():
            _fc[0] += 1
            return offp.tile([128, 16], F32, name=f"f16_{_fc[0]}", tag=f"f16_{_fc[0]}")

        xs = f16(); ys = f16()
        nc.vector.scalar_tensor_tensor(out=xs[:], in0=offx[:], scalar=4.0 * C15,
                                       in1=meshA[:], op0=OP.mult, op1=OP.add)
        nc.vector.scalar_tensor_tensor(out=ys[:], in0=offy[:], scalar=4.0 * C15,
                                       in1=meshB[:], op0=OP.mult, op1=OP.add)

        def floor_of(src):
            _fc[0] += 1
            ti = offp.tile([128, 16], I32, name=f"i16_{_fc[0]}", tag=f"i16_{_fc[0]}")
            nc.vector.tensor_copy(ti[:], src)
            tf = f16()
            nc.vector.tensor_copy(tf[:], ti[:])
            gt = f16()
            nc.vector.tensor_tensor(out=gt[:], in0=tf[:], in1=src, op=OP.is_gt)
            fl = f16()
            nc.vector.tensor_tensor(out=fl[:], in0=tf[:], in1=gt[:], op=OP.subtract)
            return fl

        x0f = floor_of(xs[:]); y0f = floor_of(ys[:])

        def in_range(v, lo, hi):
            a = f16(); b2 = f16(); r = f16()
            nc.vector.tensor_scalar(out=a[:], in0=v, scalar1=float(lo), scalar2=None,
                                    op0=OP.is_ge)
            nc.vector.tensor_scalar(out=b2[:], in0=v, scalar1=float(hi), scalar2=None,
                                    op0=OP.is_le)
            nc.vector.tensor_tensor(out=r[:], in0=a[:], in1=b2[:], op=OP.mult)
            return r

        vx0 = in_range(x0f[:], 0, 63); vx1 = in_range(x0f[:], -1, 62)
        vy0 = in_range(y0f[:], 0, 63); vy1 = in_range(y0f[:], -1, 62)
        wx1 = f16(); wy1 = f16()
        nc.vector.tensor_tensor(out=wx1[:], in0=xs[:], in1=x0f[:], op=OP.subtract)
        nc.vector.tensor_tensor(out=wy1[:], in0=ys[:], in1=y0f[:], op=OP.subtract)
        wx0m = f16(); wx1m = f16(); wy0m = f16(); wy1m = f16()
        nc.vector.scalar_tensor_tensor(out=wx0m[:], in0=wx1[:], scalar=1.0,
                                       in1=vx0[:], op0=OP.subtract, op1=OP.mult)
        nc.vector.tensor_scalar_mul(wx0m[:], wx0m[:], -1.0)
        nc.vector.tensor_tensor(out=wx1m[:], in0=wx1[:], in1=vx1[:], op=OP.mult)
        nc.vector.scalar_tensor_tensor(out=wy0m[:], in0=wy1[:], scalar=1.0,
                                       in1=vy0[:], op0=OP.subtract, op1=OP.mult)
        nc.vector.tensor_scalar_mul(wy0m[:], wy0m[:], -1.0)
        nc.vector.tensor_tensor(out=wy1m[:], in0=wy1[:], in1=vy1[:], op=OP.mult)
        nc.vector.tensor_tensor(out=Wb[:, 0:16], in0=wy0m[:], in1=wx0m[:], op=OP.mult)
        nc.vector.tensor_tensor(out=Wb[:, 16:32], in0=wy0m[:], in1=wx1m[:], op=OP.mult)
        nc.vector.tensor_tensor(out=Wb[:, 32:48], in0=wy1m[:], in1=wx0m[:], op=OP.mult)
        nc.vector.tensor_tensor(out=Wb[:, 48:64], in0=wy1m[:], in1=wx1m[:], op=OP.mult)
        xm = f16(); ym0 = f16(); ym1 = f16()
        nc.vector.tensor_scalar(out=xm[:], in0=x0f[:], scalar1=-1.0, scalar2=63.0,
                                op0=OP.max, op1=OP.min)
        nc.vector.tensor_scalar(out=ym0[:], in0=y0f[:], scalar1=0.0, scalar2=63.0,
                                op0=OP.max, op1=OP.min)
        nc.vector.tensor_scalar(out=ym1[:], in0=y0f[:], scalar1=1.0, scalar2=0.0,
                                op0=OP.add, op1=OP.max)
        nc.vector.tensor_scalar_min(ym1[:], ym1[:], 63.0)
        IDXf = offp.tile([128, 32], F32)
        nc.vector.scalar_tensor_tensor(out=IDXf[:, 0:16], in0=ym0[:], scalar=64.0,
                                       in1=xm[:], op0=OP.mult, op1=OP.add)
        nc.vector.scalar_tensor_tensor(out=IDXf[:, 16:32], in0=ym1[:], scalar=64.0,
                                       in1=xm[:], op0=OP.mult, op1=OP.add)
        nc.vector.tensor_copy(IDX[:], IDXf[:])

        psOff.release(); offp.release()
        qts_pool.release(); psQT.release(); psA.release()
        if stage < 2:
            warm_pool.release(); x_pool.release(); wq_pool.release()
            drp.release(); q_pool.release(); wkv_pool.release(); P0.release()
            nc.compile(); return nc

        # ---------------- gathers + bilinear + kvf ----------------
        gpool = tc.alloc_tile_pool(name="gp", bufs=4)
        psT = tc.alloc_tile_pool(name="psT", bufs=2, space="PSUM")
        qt_flat = qt_map[:]
        for g in range(G):
            Gt = gpool.tile([128, 512], F32, tag="G")
            for yy in range(2):
                for t in range(2):
                    col = yy * 16 + t * 8 + g
                    nc.gpsimd.indirect_dma_start(
                        out=Gt[:, (yy * 2 + t) * 128:(yy * 2 + t + 1) * 128],
                        out_offset=None, in_=qt_flat,
                        in_offset=bass.IndirectOffsetOnAxis(
                            ap=IDX[:, col:col + 1], axis=0),
                        element_offset=(g * MROWS + 1) * D)
            for t in range(2):
                acc = gpool.tile([128, D], F32, tag="acc")
                m = t * 8 + g
                nc.vector.tensor_scalar(out=acc[:], in0=Gt[:, t * 128:t * 128 + 64],
                                        scalar1=Wb[:, m:m + 1], scalar2=None,
                                        op0=OP.mult)
                for yy, xx in ((0, 1), (1, 0), (1, 1)):
                    blk = (yy * 2 + t) * 128 + xx * 64
                    wcol = (2 * yy + xx) * 16 + m
                    nc.vector.scalar_tensor_tensor(
                        out=acc[:], in0=Gt[:, blk:blk + 64],
                        scalar=Wb[:, wcol:wcol + 1], in1=acc[:],
                        op0=OP.mult, op1=OP.add)
                accb = gpool.tile([128, D], BF16, tag="accb")
                nc.vector.tensor_copy(accb[:], acc[:])
                pt = psT.tile([64, 128], BF16, tag="pt")
                nc.tensor.transpose(pt[:], accb[:], ident_bf[:])
                nc.vector.tensor_copy(
                    kvf[(g % 2) * 64:(g % 2) * 64 + 64, g // 2, t * 128:(t + 1) * 128],
                    pt[:])

        if stage < 3:
            psT.release(); gpool.release()
            warm_pool.release(); x_pool.release(); wq_pool.release()
            drp.release(); q_pool.release(); wkv_pool.release(); P0.release()
            nc.compile(); return nc
        # ---------------- k and vT ----------------
        psKV = tc.alloc_tile_pool(name="psKV", bufs=2, space="PSUM")
        for oc in range(4):
            pk = psKV.tile([128, J], F32, tag="pk")
            for c in range(4):
                nc.tensor.matmul(pk[:], wk_sb[:, c, oc * 128:(oc + 1) * 128],
                                 kvf[:, c, :], start=(c == 0), stop=(c == 3))
            nc.vector.tensor_copy(k_sb[:, oc, :], pk[:])
        nc.vector.memset(vT_sb[:], 1.0)
        for jt in range(2):
            pv = psKV.tile([128, INNER], F32, tag="pv")
            for c in range(4):
                nc.tensor.matmul(pv[:], kvf[:, c, jt * 128:(jt + 1) * 128],
                                 wv_sb[:, c, :], start=(c == 0), stop=(c == 3))
            for h in range(8):
                nc.vector.tensor_copy(vT_sb[:, jt, h * 65:h * 65 + 64],
                                      pv[:, h * 64:(h + 1) * 64])
        i0 = _wi[0]
        vdep(k_sb[:, 0, 0:1])
        vdep(vT_sb[:, 0, 0:1])
        nc.tensor.matmul(warm_t[0:2, 24:26], stg[:, i0:i0 + 2],
                         stg[:, i0:i0 + 2], start=True, stop=True)
        psKV.release(); psT.release(); gpool.release()
        warm_pool.release()
        x_pool.release(); wq_pool.release()
        if stage < 4:
            drp.release(); q_pool.release(); wkv_pool.release(); P0.release()
            nc.compile(); return nc

        # ---------------- fused attention + output projection ----------------
        # PSUM budget (8 banks): psS 2 bufs x [128,2,512]f32 (2 banks each) = 4,
        # psAV 2 x [128,4,65]f32 = 2, psTB 1 x [128,512]bf16 = 1,
        # psF 1 x [128,512]f32 = 1.
        outT_pool = tc.alloc_tile_pool(name="otp", bufs=1)
        outT_sb = outT_pool.tile([128, 4, S], BF16)
        ep = tc.alloc_tile_pool(name="ep", bufs=4)
        psS = tc.alloc_tile_pool(name="psS", bufs=2, space="PSUM")
        psAV = tc.alloc_tile_pool(name="psAV", bufs=2, space="PSUM")
        psTB = tc.alloc_tile_pool(name="psTB", bufs=1, space="PSUM")
        psF = tc.alloc_tile_pool(name="psF", bufs=1, space="PSUM")
        nrm = tc.alloc_tile_pool(name="nrm", bufs=4)
        recp = tc.alloc_tile_pool(name="recp", bufs=4)
        yev = tc.alloc_tile_pool(name="yev", bufs=3)

        E_tiles = {}
        outF_tiles = {}

        def emit_sim(u):
            """sim + exp for unit u = (blk, hp): 1024 cols of E for 2 heads."""
            blk, hp = u // 4, u % 4
            bs = blk * 1024
            E = ep.tile([128, 2, 2, 1024], BF16, tag="E")
            E_tiles[u] = E
            for jt in range(2):
                for half in range(2):
                    cs = bs + half * 512
                    ps = psS.tile([128, 2, 512], F32, tag="ps")
                    for hh in range(2):
                        nc.tensor.matmul(
                            ps[:, hh, :],
                            k_sb[hh * 64:(hh + 1) * 64, hp, jt * 128:(jt + 1) * 128],
                            q_sb[hh * 64:(hh + 1) * 64, hp, cs:cs + 512],
                            start=True, stop=True)
                    nc.scalar.activation(
                        E[:, :, jt, half * 512:(half + 1) * 512], ps[:],
                        AF.Exp, scale=SCALE)

        def emit_av_mm(u):
            """AV matmuls + normalize into outF tiles for unit u."""
            blk, hp = u // 4, u % 4
            E = E_tiles.pop(u)
            for sl in range(2):  # 512-col halves of the block
                outF = nrm.tile([128, 4, 128], BF16, tag="outF")
                outF_tiles[(u, sl)] = outF
                for hh in range(2):
                    h = 2 * hp + hh
                    pav = psAV.tile([128, 4, 65], F32, tag="pav")
                    for ck in range(4):
                        cl = sl * 512 + ck * 128
                        for jt in range(2):
                            nc.tensor.matmul(
                                pav[:, ck, :], E[:, hh, jt, cl:cl + 128],
                                vT_sb[:, jt, h * 65:(h + 1) * 65],
                                start=(jt == 0), stop=(jt == 1))
                    rec = recp.tile([128, 4], F32, tag="rec")
                    nc.vector.reciprocal(rec[:], pav[:, :, 64])
                    nc.vector.tensor_tensor(
                        out=outF[:, :, hh * 64:(hh + 1) * 64],
                        in0=pav[:, :, 0:64],
                        in1=rec[:].unsqueeze(-1).to_broadcast((128, 4, 64)),
                        op=OP.mult)

        def emit_ptb(u):
            """transpose outF into outT columns for unit u."""
            blk, hp = u // 4, u % 4
            for sl in range(2):
                sb2 = blk * 2 + sl
                outF = outF_tiles.pop((u, sl))
                ptb = psTB.tile([128, 512], BF16, tag="ptb")
                for ck in range(4):
                    nc.tensor.transpose(ptb[:, ck * 128:(ck + 1) * 128],
                                        outF[:, ck, :], ident_bf[:])
                nc.vector.tensor_copy(
                    outT_sb[:, hp, sb2 * 512:(sb2 + 1) * 512], ptb[:])

        def emit_final_chunk(sb2, oc):
            pf = psF.tile([128, 512], F32, tag="pf")
            for ic in range(4):
                nc.tensor.matmul(pf[:], wo_sb[:, ic, oc * 128:(oc + 1) * 128],
                                 outT_sb[:, ic, sb2 * 512:(sb2 + 1) * 512],
                                 start=(ic == 0), stop=(ic == 3))
            ye = yev.tile([128, 512], F32, tag="ye")
            nc.vector.tensor_scalar(out=ye[:], in0=pf[:],
                                    scalar1=bout_sb[:, oc:oc + 1], scalar2=None,
                                    op0=OP.add)
            nc.sync.dma_start(
                y_out.ap()[oc * 128:(oc + 1) * 128,
                           sb2 * 512:(sb2 + 1) * 512], ye[:])

        # software pipeline over 16 (blk, hp) units; final-projection chunks
        # of a completed block are drip-fed between later units so the
        # single-buffered psF never stalls the tensor queue.
        pending_final = []
        emit_sim(0)
        emit_sim(1)
        for u in range(16):
            emit_av_mm(u)
            if u + 2 < 16:
                emit_sim(u + 2)
            emit_ptb(u)
            if u % 4 == 3:
                blk = u // 4
                pending_final.extend(
                    (blk * 2 + sl, oc) for sl in range(2) for oc in range(4))
            for _ in range(2):
                if pending_final and u < 15:
                    emit_final_chunk(*pending_final.pop(0))
        while pending_final:
            emit_final_chunk(*pending_final.pop(0))

        yev.release(); recp.release(); nrm.release()
        psF.release(); psTB.release(); psAV.release(); psS.release(); ep.release()
        outT_pool.release()
        drp.release(); q_pool.release(); wkv_pool.release(); P0.release()
    nc.compile()
    return nc


# ---------------------------------------------------------------------------
# Public entry point: full (unsharded) inputs -> full output.
# Data-parallel over batch: image i runs on NeuronCore i (8 cores).
# ---------------------------------------------------------------------------
_NC_CACHE = {}


def _get_nc():
    if "nc" not in _NC_CACHE:
        _NC_CACHE["nc"] = build()
    return _NC_CACHE["nc"]


def kernel(x, w_q, w_off1, b_off1, w_off2, w_kv, w_out, b_out):
    from concourse.bass_utils import run_bass_kernel_spmd
    x = np.asarray(x, np.float32)
    b = x.shape[0]
    assert x.shape == (8, DIM, H, W), f"unexpected x shape {x.shape}"
    wd = prep_weights(w_q, w_off1, b_off1, w_off2, w_kv, w_out, b_out)
    in_maps = [{"x": np.ascontiguousarray(x[i].reshape(DIM, S)), **wd}
               for i in range(b)]
    nc = _get_nc()
    res = run_bass_kernel_spmd(nc, in_maps, core_ids=list(range(b)))
    out = np.stack([res.results[i]["y"].reshape(DIM, H, W) for i in range(b)])
    return out.astype(np.float32)
